# revision 1
# baseline (speedup 1.0000x reference)
"""Trainium2 Bass kernel for a 3-layer GIN encoder (gnn_message_passing).

Reference computation (per layer l):
    agg_i = sum_{j -> i} z_j          (scatter-add over edges)
    h     = z + agg                   (GIN eps=0, folded in as self-edges)
    z     = relu(relu(h @ w1 + b1) @ w2 + b2)

Distribution strategy (8 NeuronCores, SPMD single program):
  * Nodes are block-sharded: core c owns rows [c*NPC, (c+1)*NPC).
  * Edges are partitioned by destination core; the aggregation is local.
  * Each layer's full activation table z (bf16, row-major) lives in DRAM on
    every core (AllGather at layer boundaries = the halo exchange in the
    extreme case of a dense random graph).
  * Aggregation runs on TensorE as a dense matmul with the local adjacency
    count matrix:  h.T = z.T @ Aloc.T  where Aloc[dst_slot, src] counts
    edges (incl. one self-edge per node).  Aloc.T (bf16, exact small ints)
    is precomputed on the host and streamed from HBM in K-chunks; z sits in
    SBUF as 128-row chunks that serve as the stationary matmul operand.
    This replaces a per-edge SWDGE gather, whose Q7 descriptor generation
    (~9 ns/index) was measured as the bottleneck.
  * h.T (features on partitions) feeds the MLP directly.  The MLP runs on
    groups of 4 M-tiles (512 rows in the free dim) with hi/lo-split bf16
    matmuls (error ~= fp32) accumulated in fp32 PSUM.
  * Output rows are transposed back via TensorE and DMA'd out; layers 0..L-2
    are AllGathered into the next layer's activation table.
"""

import os
import sys

sys.path.insert(0, "/opt/trn_rl_repo")

import numpy as np
import ml_dtypes

BF16 = ml_dtypes.bfloat16
P = 128
NCORES = 8

# number of hi/lo product terms in the MLP matmuls:
# 3 = (w_hi*h_hi + w_hi*h_lo + w_lo*h_hi) ~ fp32 accuracy
# 1 = plain bf16
NSPLIT = 3

# adjacency K-chunks fetched per DMA (batching amortizes HWDGE issue cost)
ABATCH = 4
# adjacency K-chunks kept resident in SBUF across layers
ACACHE = 4
# cached chunks issued before the first streamed batch of each layer
CFIRST = 5

_BUILD_CACHE: dict = {}


# --------------------------------------------------------------------------
# host-side preprocessing
# --------------------------------------------------------------------------

def _config(inputs):
    x = inputs["x"]
    N, DIN = int(x.shape[0]), int(x.shape[1])
    L = 0
    while f"w1_{L}" in inputs:
        L += 1
    DH = int(inputs["w1_0"].shape[1])
    assert N % NCORES == 0
    NPC = N // NCORES
    MT = (NPC + P - 1) // P
    KC = (N + P - 1) // P
    assert DIN % P == 0 and DH % P == 0
    return dict(N=N, DIN=DIN, DH=DH, L=L, NPC=NPC, MT=MT, KC=KC)


def _prep_at(edge_index, N, NPC, MT, KC):
    """Dense transposed local adjacency per core.

    Returns at[NCORES] each [KC, 128, MT*128] bf16 with
    at[c][k, p, s] = #edges (src = k*128+p) -> (dst = c*NPC + s), plus the
    identity (self-edge).  Src rows beyond N and dst slots beyond NPC are 0.
    """
    src = np.asarray(edge_index[0], dtype=np.int64)
    dst = np.asarray(edge_index[1], dtype=np.int64)
    self_ix = np.arange(N, dtype=np.int64)
    allsrc = np.concatenate([src, self_ix])
    alldst = np.concatenate([dst, self_ix])

    core = alldst // NPC
    gslot = core * (MT * P) + (alldst - core * NPC)

    at = np.zeros((KC * P, NCORES * MT * P), np.float32)
    np.add.at(at, (allsrc, gslot), 1.0)
    at_bf = at.astype(BF16)
    at_u8 = at.astype(np.uint8)

    CA = min(ACACHE, KC)
    NSB = -(-(KC - CA) // ABATCH)          # streamed batches (padded)
    CAB = -(-CA // ABATCH)                 # cache-load batches
    KPAD = CA + NSB * ABATCH

    def batched(arr, lo, hi, nb):
        # [chunks, P, MT*P] -> [nb, P, ABATCH*MT*P], batch-contiguous per
        # partition so each stream DMA moves ABATCH*MT*P*esz contiguous
        # bytes per partition
        sl = arr[lo:hi]
        pad = nb * ABATCH - (hi - lo)
        if pad:
            sl = np.concatenate(
                [sl, np.zeros((pad,) + sl.shape[1:], sl.dtype)], axis=0)
        return np.ascontiguousarray(
            sl.reshape(nb, ABATCH, P, MT * P)
              .transpose(0, 2, 1, 3)
              .reshape(nb, P, ABATCH * MT * P))

    aca, ats, at8s = [], [], []
    for c in range(NCORES):
        sl_bf = at_bf[:, c * MT * P:(c + 1) * MT * P].reshape(KC, P, MT * P)
        sl_u8 = at_u8[:, c * MT * P:(c + 1) * MT * P].reshape(KC, P, MT * P)
        aca.append(batched(sl_bf, 0, CA, CAB))
        ats.append(batched(sl_bf, CA, KC, NSB))
        at8s.append(batched(sl_u8, CA, KC, NSB))
    return aca, ats, at8s


# --------------------------------------------------------------------------
# bass program
# --------------------------------------------------------------------------

def _build(N, DIN, DH, L, NPC, MT, KC):
    from concourse import bacc, mybir, tile

    f32 = mybir.dt.float32
    bf = mybir.dt.bfloat16
    SUB = mybir.AluOpType.subtract
    RELU = mybir.ActivationFunctionType.Relu

    NKT2 = DH // P  # K/M tiles of the hidden dim (2)
    GROUPS = [(g0, min(4, MT - g0)) for g0 in range(0, MT, 4)]
    # slot groups for the aggregation matmul free dim (<=512 per PSUM bank)
    NG = [(n0, min(512, MT * P - n0)) for n0 in range(0, MT * P, 512)]
    NFULL = (N // P) * P
    NREM = N - NFULL

    nc = bacc.Bacc(num_devices=NCORES)

    xin = nc.dram_tensor("x_bf", [N, DIN], bf, kind="ExternalInput")
    CA = min(ACACHE, KC)
    NSB = -(-(KC - CA) // ABATCH)
    CAB = -(-CA // ABATCH)
    acain = nc.dram_tensor("aca", [CAB, P, ABATCH * MT * P], bf,
                           kind="ExternalInput")
    atsin = nc.dram_tensor("ats", [NSB, P, ABATCH * MT * P], bf,
                           kind="ExternalInput")
    at8in = nc.dram_tensor("at8s", [NSB, P, ABATCH * MT * P], mybir.dt.uint8,
                           kind="ExternalInput")
    identbin = nc.dram_tensor("identb", [P, P], bf, kind="ExternalInput")
    identfin = nc.dram_tensor("identf", [P, P], f32, kind="ExternalInput")
    win = {}
    for l in range(L):
        din = DIN if l == 0 else DH
        for nm, shp in [
            ("w1h", [din, DH]), ("w1l", [din, DH]),
            ("w2h", [DH, DH]), ("w2l", [DH, DH]),
        ]:
            win[(nm, l)] = nc.dram_tensor(f"{nm}_{l}", shp, bf, kind="ExternalInput")
        for nm in ("b1", "b2"):
            win[(nm, l)] = nc.dram_tensor(f"{nm}_{l}", [DH, 1], f32, kind="ExternalInput")
    zout = nc.dram_tensor("zout", [NPC, DH], f32, kind="ExternalOutput")

    with tile.TileContext(nc) as tc:
        with tc.tile_pool(name="const", bufs=1) as cp, \
             tc.tile_pool(name="atpool", bufs=6) as atp, \
             tc.tile_pool(name="a8pool", bufs=2) as a8p, \
             tc.tile_pool(name="zsbpool", bufs=1) as zsp, \
             tc.tile_pool(name="hpool", bufs=1) as hp, \
             tc.tile_pool(name="spool", bufs=2) as sp, \
             tc.tile_pool(name="zpool", bufs=1) as zp, \
             tc.tile_pool(name="zrpool", bufs=3) as zrp, \
             tc.tile_pool(name="hpsum", bufs=1, space="PSUM") as hpsum, \
             tc.tile_pool(name="mlppsum", bufs=2, space="PSUM") as mlppool, \
             tc.tile_pool(name="drampool", bufs=1, space="DRAM") as dp:

            # ---------------- resident constants ----------------
            identb_t = cp.tile([P, P], bf, name="identb_t")
            nc.gpsimd.dma_start(out=identb_t[:], in_=identbin[:, :])
            identf_t = cp.tile([P, P], f32, name="identf_t")
            nc.gpsimd.dma_start(out=identf_t[:], in_=identfin[:, :])

            # resident head of the adjacency (reused by all layers)
            acache = cp.tile([P, CA * MT * P], bf, name="acache")
            for b in range(CAB):
                nc.gpsimd.dma_start(
                    out=acache[:, b * ABATCH * MT * P:(b + 1) * ABATCH * MT * P],
                    in_=acain[b, :, :])

            wt = {}
            for l in range(L):
                din = DIN if l == 0 else DH
                nkt = din // P
                for nm in ("w1h", "w1l"):
                    t = cp.tile([P, nkt * DH], bf, name=f"{nm}{l}_t")
                    for kt in range(nkt):
                        nc.gpsimd.dma_start(
                            out=t[:, kt * DH:(kt + 1) * DH],
                            in_=win[(nm, l)][kt * P:(kt + 1) * P, :])
                    wt[(nm, l)] = t
                for nm in ("w2h", "w2l"):
                    t = cp.tile([P, NKT2 * DH], bf, name=f"{nm}{l}_t")
                    for kt in range(NKT2):
                        nc.gpsimd.dma_start(
                            out=t[:, kt * DH:(kt + 1) * DH],
                            in_=win[(nm, l)][kt * P:(kt + 1) * P, :])
                    wt[(nm, l)] = t
                for nm in ("b1", "b2"):
                    t = cp.tile([P, NKT2], f32, name=f"{nm}{l}_t")
                    for mo in range(NKT2):
                        nc.gpsimd.dma_start(
                            out=t[:, mo:mo + 1],
                            in_=win[(nm, l)][mo * P:(mo + 1) * P, :])
                    wt[(nm, l)] = t

            # layer-boundary activation tables
            zloc = [dp.tile([NPC, DH], bf, name=f"zloc{l}") for l in range(L - 1)]
            zfull = [dp.tile([N, DH], bf, name=f"zfull{l}", addr_space="Shared")
                     for l in range(L - 1)]


            # ---------------- layers ----------------
            for l in range(L):
                din = DIN if l == 0 else DH
                nkt = din // P
                last = (l == L - 1)
                table = xin[:, :] if l == 0 else zfull[l - 1][:, :]

                # activation table -> SBUF, chunked [128, KC*din]:
                # zsb[p, k*din+f] = z[k*128+p, f]; split into pieces so the
                # K-loop matmuls can start before the whole table landed
                zsb = zsp.tile([P, KC * din], bf, name=f"zsb_{l}", tag="zsb")
                KFULL = N // P
                ZPIECE = 8
                for z0 in range(0, KFULL, ZPIECE):
                    z1 = min(KFULL, z0 + ZPIECE)
                    nc.scalar.dma_start(
                        out=zsb[:, z0 * din: z1 * din]
                            .rearrange("p (k f) -> p k f", f=din),
                        in_=table[z0 * P: z1 * P, :]
                            .rearrange("(k p) f -> p k f", p=P))
                if NREM:
                    nc.vector.memset(zsb[:, KFULL * din:], 0.0)
                    nc.scalar.dma_start(
                        out=zsb[:NREM, KFULL * din:],
                        in_=table[NFULL:, :])

                # --- aggregation: h.T = z.T @ Aloc.T  (PSUM-accumulated)
                hps = [hpsum.tile([P, len(NG) * 512], f32,
                                  name=f"hps{mf}_{l}", tag=f"hps{mf}")
                       for mf in range(nkt)]
                def agg_mms(k, rhs_tile, rhs_off, first, final):
                    for mf in range(nkt):
                        for gi, (n0, nn) in enumerate(NG):
                            nc.tensor.matmul(
                                out=hps[mf][:, gi * 512: gi * 512 + nn],
                                lhsT=zsb[:, k * din + mf * P: k * din + (mf + 1) * P],
                                rhs=rhs_tile[:, rhs_off + n0: rhs_off + n0 + nn],
                                start=first,
                                stop=final,
                            )

                # Interleave SBUF-cached chunks between streamed batches so
                # TensorE never stalls on the A.T stream (keeps HAM warm).
                # A few cached chunks go FIRST: right after an AllGather they
                # only need the first zsb piece, no streamed data.
                cached = list(range(CA))
                head, rest = cached[:CFIRST], cached[CFIRST:]
                seq = [("C", k) for k in head]
                nb = max(1, NSB)
                per = [len(rest) * (bi + 1) // nb for bi in range(nb)]
                ci = 0
                for bi in range(NSB):
                    seq.append(("S", bi))
                    while ci < per[bi]:
                        seq.append(("C", rest[ci]))
                        ci += 1
                while ci < len(rest):
                    seq.append(("C", rest[ci]))
                    ci += 1

                nchunks = KC
                done = 0
                for kind, payload in seq:
                    if kind == "S":
                        b = payload
                        ks = [CA + b * ABATCH + j for j in range(ABATCH)
                              if CA + b * ABATCH + j < KC]
                        at_t = atp.tile([P, ABATCH * MT * P], bf,
                                        name=f"at_{l}_{b}", tag="at")
                        if l == 0:
                            # layer 0 is DMA-bound: stream uint8, cast to
                            # bf16 on the otherwise-idle DVE/ACT engines;
                            # 5/8-3/8 split balances them (ACT copy is
                            # ~1.6x slower than DVE)
                            at8_t = a8p.tile([P, ABATCH * MT * P],
                                             mybir.dt.uint8,
                                             name=f"at8_{l}_{b}", tag="at8")
                            nc.sync.dma_start(out=at8_t[:], in_=at8in[b, :, :])
                            cut = ABATCH * MT * P * 5 // 8
                            nc.vector.tensor_copy(
                                out=at_t[:, :cut], in_=at8_t[:, :cut])
                            nc.scalar.activation(
                                out=at_t[:, cut:], in_=at8_t[:, cut:],
                                func=mybir.ActivationFunctionType.Copy)
                        else:
                            nc.sync.dma_start(out=at_t[:], in_=atsin[b, :, :])
                        for k in ks:
                            agg_mms(k, at_t, (k - CA - b * ABATCH) * MT * P,
                                    done == 0, done == nchunks - 1)
                            done += 1
                    else:
                        k = payload
                        agg_mms(k, acache, k * MT * P,
                                done == 0, done == nchunks - 1)
                        done += 1

                # --- split h.T into hi/lo bf16
                hhi = [hp.tile([P, MT * P], bf, name=f"hhi{mf}_{l}", tag=f"hhi{mf}")
                       for mf in range(nkt)]
                hlo = [hp.tile([P, MT * P], bf, name=f"hlo{mf}_{l}", tag=f"hlo{mf}")
                       for mf in range(nkt)]
                for mf in range(nkt):
                    for gi, (n0, nn) in enumerate(NG):
                        nc.vector.tensor_copy(
                            out=hhi[mf][:, n0:n0 + nn],
                            in_=hps[mf][:, gi * 512: gi * 512 + nn])
                        nc.vector.tensor_tensor(
                            out=hlo[mf][:, n0:n0 + nn],
                            in0=hps[mf][:, gi * 512: gi * 512 + nn],
                            in1=hhi[mf][:, n0:n0 + nn],
                            op=SUB)

                # --- MLP over groups of 4 M-tiles (512-row free dim)
                zT = [zp.tile([P, MT * P], f32 if last else bf,
                              name=f"zT{mo}_{l}",
                              tag=f"zT{mo}{'f' if last else 'b'}")
                      for mo in range(NKT2)]
                for (g0, gm) in GROUPS:
                    rows = gm * P
                    r0 = g0 * P
                    combos1 = [("w1h", hhi), ("w1h", hlo), ("w1l", hhi)][:NSPLIT]
                    s1h, s1l = [], []
                    for mo in range(NKT2):
                        p1 = mlppool.tile([P, 512], f32,
                                          name=f"p1_{l}_{g0}_{mo}", tag="mlp")
                        tot = len(combos1) * nkt
                        step = 0
                        for (wn, ht) in combos1:
                            for kt in range(nkt):
                                nc.tensor.matmul(
                                    out=p1[:, :rows],
                                    lhsT=wt[(wn, l)][:, kt * DH + mo * P: kt * DH + (mo + 1) * P],
                                    rhs=ht[kt][:, r0:r0 + rows],
                                    start=(step == 0), stop=(step == tot - 1))
                                step += 1
                        s1f = sp.tile([P, 512], f32, name=f"s1f_{l}_{g0}_{mo}", tag="s1f")
                        nc.scalar.activation(
                            out=s1f[:, :rows], in_=p1[:, :rows], func=RELU,
                            bias=wt[("b1", l)][:, mo:mo + 1])
                        sh = sp.tile([P, 512], bf, name=f"s1h_{l}_{g0}_{mo}", tag=f"s1h{mo}")
                        nc.vector.tensor_copy(out=sh[:, :rows], in_=s1f[:, :rows])
                        sl = sp.tile([P, 512], bf, name=f"s1l_{l}_{g0}_{mo}", tag=f"s1l{mo}")
                        nc.vector.tensor_tensor(
                            out=sl[:, :rows], in0=s1f[:, :rows], in1=sh[:, :rows], op=SUB)
                        s1h.append(sh)
                        s1l.append(sl)
                    combos2 = [("w2h", s1h), ("w2h", s1l), ("w2l", s1h)][:NSPLIT]
                    for mo in range(NKT2):
                        p2 = mlppool.tile([P, 512], f32,
                                          name=f"p2_{l}_{g0}_{mo}", tag="mlp")
                        tot = len(combos2) * NKT2
                        step = 0
                        for (wn, st) in combos2:
                            for kt in range(NKT2):
                                nc.tensor.matmul(
                                    out=p2[:, :rows],
                                    lhsT=wt[(wn, l)][:, kt * DH + mo * P: kt * DH + (mo + 1) * P],
                                    rhs=st[kt][:, :rows],
                                    start=(step == 0), stop=(step == tot - 1))
                                step += 1
                        nc.scalar.activation(
                            out=zT[mo][:, r0:r0 + rows], in_=p2[:, :rows], func=RELU,
                            bias=wt[("b2", l)][:, mo:mo + 1])

                    # transpose this group's M-tiles back to row-major + store
                    ident = identf_t if last else identb_t
                    for m in range(g0, g0 + gm):
                        rows_m = min(P, NPC - m * P)
                        tp = mlppool.tile([P, NKT2 * P], f32 if last else bf,
                                          name=f"tp_{l}_{m}", tag="mlp")
                        for mo in range(NKT2):
                            nc.tensor.transpose(
                                out=tp[:, mo * P:(mo + 1) * P],
                                in_=zT[mo][:, m * P:(m + 1) * P],
                                identity=ident[:])
                        zr = zrp.tile([P, NKT2 * P], f32 if last else bf,
                                      name=f"zr_{l}_{m}", tag="zr")
                        nc.vector.tensor_copy(out=zr[:], in_=tp[:])
                        dst = zout if last else zloc[l]
                        nc.sync.dma_start(
                            out=dst[m * P: m * P + rows_m, :],
                            in_=zr[:rows_m, :])

                if not last:
                    nc.gpsimd.collective_compute(
                        "AllGather",
                        mybir.AluOpType.bypass,
                        replica_groups=[list(range(NCORES))],
                        ins=[zloc[l][:, :].opt()],
                        outs=[zfull[l][:, :].opt()],
                    )

    # populates extended-inst ISA bytes + inserts GPSIMD library loads
    nc.compile()
    return nc


# --------------------------------------------------------------------------
# entry point
# --------------------------------------------------------------------------

def _make_in_maps(inputs, cfg, aca, ats, at8s):
    DH, L = cfg["DH"], cfg["L"]
    x_bf = np.ascontiguousarray(np.asarray(inputs["x"], dtype=np.float32)).astype(BF16)
    identb = np.eye(P, dtype=np.float32).astype(BF16)
    identf = np.eye(P, dtype=np.float32)

    shared = {"x_bf": x_bf, "identb": identb, "identf": identf}
    for l in range(L):
        w1 = np.asarray(inputs[f"w1_{l}"], dtype=np.float32)
        w2 = np.asarray(inputs[f"w2_{l}"], dtype=np.float32)
        w1h = w1.astype(BF16)
        w2h = w2.astype(BF16)
        shared[f"w1h_{l}"] = w1h
        shared[f"w1l_{l}"] = (w1 - w1h.astype(np.float32)).astype(BF16)
        shared[f"w2h_{l}"] = w2h
        shared[f"w2l_{l}"] = (w2 - w2h.astype(np.float32)).astype(BF16)
        shared[f"b1_{l}"] = np.asarray(
            inputs[f"b1_{l}"], dtype=np.float32).reshape(DH, 1)
        shared[f"b2_{l}"] = np.asarray(
            inputs[f"b2_{l}"], dtype=np.float32).reshape(DH, 1)

    in_maps = []
    for c in range(NCORES):
        m = dict(shared)
        m["aca"] = aca[c]
        m["ats"] = ats[c]
        m["at8s"] = at8s[c]
        in_maps.append(m)
    return in_maps


def get_program(inputs):
    """Build (or fetch cached) the bass program + per-core input maps."""
    cfg = _config(inputs)
    aca, ats, at8s = _prep_at(
        inputs["edge_index"], cfg["N"], cfg["NPC"], cfg["MT"], cfg["KC"])
    key = (cfg["N"], cfg["DIN"], cfg["DH"], cfg["L"], NSPLIT)
    if key not in _BUILD_CACHE:
        _BUILD_CACHE[key] = _build(
            cfg["N"], cfg["DIN"], cfg["DH"], cfg["L"],
            cfg["NPC"], cfg["MT"], cfg["KC"])
    nc = _BUILD_CACHE[key]
    in_maps = _make_in_maps(inputs, cfg, aca, ats, at8s)
    return nc, in_maps, cfg


def kernel(**inputs):
    nc, in_maps, cfg = get_program(inputs)

    if os.environ.get("KERNEL_USE_SIM"):
        from concourse.bass_interp import MultiCoreSim
        sim = MultiCoreSim(nc, num_cores=NCORES)
        cores = list(sim.cores.values())
        for cid, cs in enumerate(cores):
            for name, val in in_maps[cid].items():
                cs.tensor(name)[:] = val
        sim.simulate(check_with_hw=False)
        parts = [np.asarray(cs.tensor("zout")) for cs in cores]
    else:
        from concourse import bass_utils
        res = bass_utils.run_bass_kernel_spmd(
            nc, in_maps, core_ids=list(range(NCORES)),
            trace=bool(os.environ.get("KERNEL_TRACE")),
        )
        kernel.last_results = res
        parts = [res.results[c]["zout"] for c in range(NCORES)]

    out = np.concatenate(parts, axis=0).astype(np.float32)
    return out



# revision 4
# speedup vs baseline: 1.5088x; 1.5088x over previous
"""Trainium2 Bass kernel for a 3-layer GIN encoder (gnn_message_passing).

Reference computation (per layer l):
    agg_i = sum_{j -> i} z_j          (scatter-add over edges)
    h     = z + agg                   (GIN eps=0, folded in as self-edges)
    z     = relu(relu(h @ w1 + b1) @ w2 + b2)

Distribution strategy (8 NeuronCores, SPMD single program):
  * Nodes block-sharded: core c owns rows [c*NPC, (c+1)*NPC); edges
    partitioned by destination core so aggregation is local; each layer's
    full activation table is AllGathered (halo exchange for a dense random
    graph).
  * Aggregation as a dense matmul with the local adjacency count matrix in
    fp8_e4m3 (counts are small ints -> exact).  The z table is also fp8
    (measured end-to-end rel err ~5e-3, bar is 2e-2), which enables
    MatmulPerfMode.DoubleRow: K=256 per instruction at 0.5 cycles/column =
    4x bf16 throughput.
  * The whole per-core adjacency (80 chunks x 1280 slots x 1B = 100KB per
    partition) stays resident in SBUF: streamed from HBM once during layer
    0, read for free in layers 1-2.
  * MLP in bf16 with hi/lo splits (3 product terms ~ fp32 accuracy),
    PSUM-accumulated; outputs transposed back via TensorE and stored fp8
    (f32 for the final layer).
"""

import os
import sys

sys.path.insert(0, "/opt/trn_rl_repo")

import numpy as np
import ml_dtypes

BF16 = ml_dtypes.bfloat16
FP8 = ml_dtypes.float8_e4m3  # TRN fp8e4 (max 240)
P = 128
NCORES = 8

# hi/lo product terms in the MLP matmuls (3 ~ fp32 accuracy)
NSPLIT = 3
# adjacency chunks fetched per stream DMA during layer 0
ABATCH = 4
# zsb chunks per load DMA piece
ZPIECE = 8

_BUILD_CACHE: dict = {}


# --------------------------------------------------------------------------
# host-side preprocessing
# --------------------------------------------------------------------------

def _config(inputs):
    x = inputs["x"]
    N, DIN = int(x.shape[0]), int(x.shape[1])
    L = 0
    while f"w1_{L}" in inputs:
        L += 1
    DH = int(inputs["w1_0"].shape[1])
    assert N % NCORES == 0
    NPC = N // NCORES
    MT = (NPC + P - 1) // P
    KC = (N + P - 1) // P
    KC2 = KC + (KC & 1)  # padded even chunk count for DoubleRow pairs
    assert DIN % P == 0 and DH % P == 0
    return dict(N=N, DIN=DIN, DH=DH, L=L, NPC=NPC, MT=MT, KC=KC, KC2=KC2)


def _prep_a8(edge_index, N, NPC, MT, KC2):
    """Dense transposed local adjacency per core, fp8, stream-batched.

    Returns a8[c] of shape [KC2//ABATCH, P, ABATCH*MT*P] fp8 with
    a8[c][b, p, j*MT*P + s] = #edges (src = (ABATCH*b+j)*128+p) ->
    (dst = c*NPC + s), plus one self-edge per node.
    """
    src = np.asarray(edge_index[0], dtype=np.int64)
    dst = np.asarray(edge_index[1], dtype=np.int64)
    self_ix = np.arange(N, dtype=np.int64)
    allsrc = np.concatenate([src, self_ix])
    alldst = np.concatenate([dst, self_ix])

    core = alldst // NPC
    gslot = core * (MT * P) + (alldst - core * NPC)

    at = np.zeros((KC2 * P, NCORES * MT * P), np.float32)
    np.add.at(at, (allsrc, gslot), 1.0)
    at8 = at.astype(FP8)

    NB = KC2 // ABATCH
    a8 = []
    for c in range(NCORES):
        sl = at8[:, c * MT * P:(c + 1) * MT * P]
        a8.append(np.ascontiguousarray(
            sl.reshape(NB, ABATCH, P, MT * P)
              .transpose(0, 2, 1, 3)
              .reshape(NB, P, ABATCH * MT * P)))
    return a8


def _prep_xz(x, DIN, KC2):
    """x in zsb layout: xz[p, k*DIN+f] = x[k*128+p, f], zero padded, fp8."""
    N = x.shape[0]
    xf = np.zeros((KC2 * P, DIN), np.float32)
    xf[:N] = np.asarray(x, dtype=np.float32)
    xz = xf.reshape(KC2, P, DIN).transpose(1, 0, 2).reshape(P, KC2 * DIN)
    return np.ascontiguousarray(xz).astype(FP8)


# --------------------------------------------------------------------------
# bass program
# --------------------------------------------------------------------------

def _build(N, DIN, DH, L, NPC, MT, KC, KC2):
    from concourse import bacc, mybir, tile

    f32 = mybir.dt.float32
    bf = mybir.dt.bfloat16
    f8 = mybir.dt.float8e4
    SUB = mybir.AluOpType.subtract
    RELU = mybir.ActivationFunctionType.Relu
    DR = mybir.MatmulPerfMode.DoubleRow

    NKT2 = DH // P            # K/M tiles of the hidden dim (2)
    MTP = MT * P              # dst slots per core (1280)
    PAIRS = KC2 // 2
    NB = KC2 // ABATCH        # layer-0 stream batches
    # dst-slot groups; aligned with MLP M-tile groups of 4 (512 rows)
    NG = [(n0, min(512, MTP - n0)) for n0 in range(0, MTP, 512)]
    GROUPS = [(g0, min(4, MT - g0)) for g0 in range(0, MT, 4)]
    KFULL = N // P
    NREM = N - KFULL * P

    nc = bacc.Bacc(num_devices=NCORES)

    xzin = nc.dram_tensor("xz", [P, KC2 * DIN], f8, kind="ExternalInput")
    a8in = nc.dram_tensor("a8", [NB, P, ABATCH * MTP], f8, kind="ExternalInput")
    identbin = nc.dram_tensor("identb", [P, P], bf, kind="ExternalInput")
    identfin = nc.dram_tensor("identf", [P, P], f32, kind="ExternalInput")
    win = {}
    for l in range(L):
        din = DIN if l == 0 else DH
        for nm, shp in [
            ("w1h", [din, DH]), ("w1l", [din, DH]),
            ("w2h", [DH, DH]), ("w2l", [DH, DH]),
        ]:
            win[(nm, l)] = nc.dram_tensor(f"{nm}_{l}", shp, bf, kind="ExternalInput")
        for nm in ("b1", "b2"):
            win[(nm, l)] = nc.dram_tensor(f"{nm}_{l}", [DH, 1], f32, kind="ExternalInput")
    zout = nc.dram_tensor("zout", [NPC, DH], f32, kind="ExternalOutput")

    with tile.TileContext(nc) as tc:
        with tc.tile_pool(name="const", bufs=1) as cp, \
             tc.tile_pool(name="zsbpool", bufs=1) as zsp, \
             tc.tile_pool(name="hpool", bufs=1) as hp, \
             tc.tile_pool(name="spool", bufs=2) as sp, \
             tc.tile_pool(name="zpool", bufs=1) as zp, \
             tc.tile_pool(name="zrpool", bufs=3) as zrp, \
             tc.tile_pool(name="hpsum", bufs=1, space="PSUM") as hpsum, \
             tc.tile_pool(name="mlppsum", bufs=2, space="PSUM") as mlppool, \
             tc.tile_pool(name="drampool", bufs=1, space="DRAM") as dp:

            # ---------------- resident constants ----------------
            identb_t = cp.tile([P, P], bf, name="identb_t")
            nc.gpsimd.dma_start(out=identb_t[:], in_=identbin[:, :])
            identf_t = cp.tile([P, P], f32, name="identf_t")
            nc.gpsimd.dma_start(out=identf_t[:], in_=identfin[:, :])

            wt = {}
            for l in range(L):
                din = DIN if l == 0 else DH
                nkt = din // P
                for nm, nk in (("w1h", nkt), ("w1l", nkt),
                               ("w2h", NKT2), ("w2l", NKT2)):
                    t = cp.tile([P, nk * DH], bf, name=f"{nm}{l}_t")
                    for kt in range(nk):
                        nc.gpsimd.dma_start(
                            out=t[:, kt * DH:(kt + 1) * DH],
                            in_=win[(nm, l)][kt * P:(kt + 1) * P, :])
                    wt[(nm, l)] = t
                for nm in ("b1", "b2"):
                    t = cp.tile([P, NKT2], f32, name=f"{nm}{l}_t")
                    for mo in range(NKT2):
                        nc.gpsimd.dma_start(
                            out=t[:, mo:mo + 1],
                            in_=win[(nm, l)][mo * P:(mo + 1) * P, :])
                    wt[(nm, l)] = t

            # resident adjacency: the whole per-core A.T in fp8
            acache = cp.tile([P, KC2, MTP], f8, name="acache")

            # layer-boundary activation tables (fp8)
            zloc = [dp.tile([NPC, DH], f8, name=f"zloc{l}") for l in range(L - 1)]
            zfull = [dp.tile([N, DH], f8, name=f"zfull{l}", addr_space="Shared")
                     for l in range(L - 1)]

            # ---------------- layers ----------------
            for l in range(L):
                din = DIN if l == 0 else DH
                nkt = din // P
                last = (l == L - 1)

                # activation table -> SBUF, node-major chunks:
                # zsb[p, k, f] = z[k*128+p, f]
                zsb = zsp.tile([P, KC2, din], f8, name=f"zsb_{l}", tag="zsb")
                if l == 0:
                    nc.scalar.dma_start(
                        out=zsb[:, :, :].rearrange("p k f -> p (k f)"),
                        in_=xzin[:, :])
                else:
                    table = zfull[l - 1]
                    nc.vector.memset(zsb[:, KFULL:, :], 0.0)
                    for z0 in range(0, KFULL, ZPIECE):
                        z1 = min(KFULL, z0 + ZPIECE)
                        nc.scalar.dma_start(
                            out=zsb[:, z0:z1, :],
                            in_=table[z0 * P: z1 * P, :]
                                .rearrange("(k p) f -> p k f", p=P))
                    if NREM:
                        nc.scalar.dma_start(
                            out=zsb[:NREM, KFULL:KFULL + 1, :],
                            in_=table[KFULL * P:, :]
                                .rearrange("(k p) f -> p k f", p=NREM))

                # --- aggregation: h.T = z.T @ Aloc.T, fp8 DoubleRow pairs,
                # k-outer so each stationary zsb slice is loaded once
                hps = [hpsum.tile([P, nkt * 512], f32,
                                  name=f"hps{gi}_{l}", tag=f"hps{gi}")
                       for gi in range(len(NG))]
                for p in range(PAIRS):
                    if l == 0 and p % (ABATCH // 2) == 0:
                        b = p // (ABATCH // 2)
                        nc.sync.dma_start(
                            out=acache[:, b * ABATCH:(b + 1) * ABATCH, :]
                                .rearrange("p k s -> p (k s)"),
                            in_=a8in[b, :, :])
                    for mf in range(nkt):
                        for gi, (n0, nn) in enumerate(NG):
                            nc.tensor.matmul(
                                out=hps[gi][:, mf * 512: mf * 512 + nn],
                                lhsT=zsb[:, 2 * p:2 * p + 2,
                                         mf * P:(mf + 1) * P],
                                rhs=acache[:, 2 * p:2 * p + 2, n0:n0 + nn],
                                start=(p == 0),
                                stop=(p == PAIRS - 1),
                                perf_mode=DR,
                            )

                # --- MLP per dst group (aligned with NG: 512 rows each)
                hhi = [hp.tile([P, MTP], bf, name=f"hhi{mf}_{l}", tag=f"hhi{mf}")
                       for mf in range(nkt)]
                hlo = [hp.tile([P, MTP], bf, name=f"hlo{mf}_{l}", tag=f"hlo{mf}")
                       for mf in range(nkt)]
                zT = [zp.tile([P, MTP], f32 if last else bf,
                              name=f"zT{mo}_{l}",
                              tag=f"zT{mo}{'f' if last else 'b'}")
                      for mo in range(NKT2)]
                for gi, (g0, gm) in enumerate(GROUPS):
                    rows = gm * P
                    r0 = g0 * P
                    # split this group's h into hi/lo bf16
                    for mf in range(nkt):
                        nc.vector.tensor_copy(
                            out=hhi[mf][:, r0:r0 + rows],
                            in_=hps[gi][:, mf * 512: mf * 512 + rows])
                        nc.vector.tensor_tensor(
                            out=hlo[mf][:, r0:r0 + rows],
                            in0=hps[gi][:, mf * 512: mf * 512 + rows],
                            in1=hhi[mf][:, r0:r0 + rows],
                            op=SUB)

                    combos1 = [("w1h", hhi), ("w1h", hlo), ("w1l", hhi)][:NSPLIT]
                    s1h, s1l = [], []
                    for mo in range(NKT2):
                        p1 = mlppool.tile([P, 512], f32,
                                          name=f"p1_{l}_{g0}_{mo}", tag="mlp")
                        tot = len(combos1) * nkt
                        step = 0
                        for (wn, ht) in combos1:
                            for kt in range(nkt):
                                nc.tensor.matmul(
                                    out=p1[:, :rows],
                                    lhsT=wt[(wn, l)][:, kt * DH + mo * P: kt * DH + (mo + 1) * P],
                                    rhs=ht[kt][:, r0:r0 + rows],
                                    start=(step == 0), stop=(step == tot - 1))
                                step += 1
                        s1f = sp.tile([P, 512], f32, name=f"s1f_{l}_{g0}_{mo}", tag="s1f")
                        nc.scalar.activation(
                            out=s1f[:, :rows], in_=p1[:, :rows], func=RELU,
                            bias=wt[("b1", l)][:, mo:mo + 1])
                        sh = sp.tile([P, 512], bf, name=f"s1h_{l}_{g0}_{mo}", tag=f"s1h{mo}")
                        nc.vector.tensor_copy(out=sh[:, :rows], in_=s1f[:, :rows])
                        sl = sp.tile([P, 512], bf, name=f"s1l_{l}_{g0}_{mo}", tag=f"s1l{mo}")
                        nc.vector.tensor_tensor(
                            out=sl[:, :rows], in0=s1f[:, :rows], in1=sh[:, :rows], op=SUB)
                        s1h.append(sh)
                        s1l.append(sl)
                    combos2 = [("w2h", s1h), ("w2h", s1l), ("w2l", s1h)][:NSPLIT]
                    for mo in range(NKT2):
                        p2 = mlppool.tile([P, 512], f32,
                                          name=f"p2_{l}_{g0}_{mo}", tag="mlp")
                        tot = len(combos2) * NKT2
                        step = 0
                        for (wn, st) in combos2:
                            for kt in range(NKT2):
                                nc.tensor.matmul(
                                    out=p2[:, :rows],
                                    lhsT=wt[(wn, l)][:, kt * DH + mo * P: kt * DH + (mo + 1) * P],
                                    rhs=st[kt][:, :rows],
                                    start=(step == 0), stop=(step == tot - 1))
                                step += 1
                        nc.scalar.activation(
                            out=zT[mo][:, r0:r0 + rows], in_=p2[:, :rows], func=RELU,
                            bias=wt[("b2", l)][:, mo:mo + 1])

                    # transpose back to row-major + store
                    ident = identf_t if last else identb_t
                    for m in range(g0, g0 + gm):
                        rows_m = min(P, NPC - m * P)
                        tp = mlppool.tile([P, NKT2 * P], f32 if last else bf,
                                          name=f"tp_{l}_{m}", tag="mlp")
                        for mo in range(NKT2):
                            nc.tensor.transpose(
                                out=tp[:, mo * P:(mo + 1) * P],
                                in_=zT[mo][:, m * P:(m + 1) * P],
                                identity=ident[:])
                        zr = zrp.tile([P, NKT2 * P], f32 if last else f8,
                                      name=f"zr_{l}_{m}", tag="zr")
                        nc.vector.tensor_copy(out=zr[:], in_=tp[:])
                        dst = zout if last else zloc[l]
                        nc.sync.dma_start(
                            out=dst[m * P: m * P + rows_m, :],
                            in_=zr[:rows_m, :])

                if not last:
                    nc.gpsimd.collective_compute(
                        "AllGather",
                        mybir.AluOpType.bypass,
                        replica_groups=[list(range(NCORES))],
                        ins=[zloc[l][:, :].opt()],
                        outs=[zfull[l][:, :].opt()],
                    )

    nc.compile()
    return nc


# --------------------------------------------------------------------------
# entry point
# --------------------------------------------------------------------------

def _make_in_maps(inputs, cfg, a8):
    DIN, DH, L, KC2 = cfg["DIN"], cfg["DH"], cfg["L"], cfg["KC2"]
    xz = _prep_xz(inputs["x"], DIN, KC2)
    identb = np.eye(P, dtype=np.float32).astype(BF16)
    identf = np.eye(P, dtype=np.float32)

    shared = {"xz": xz, "identb": identb, "identf": identf}
    for l in range(L):
        w1 = np.asarray(inputs[f"w1_{l}"], dtype=np.float32)
        w2 = np.asarray(inputs[f"w2_{l}"], dtype=np.float32)
        w1h = w1.astype(BF16)
        w2h = w2.astype(BF16)
        shared[f"w1h_{l}"] = w1h
        shared[f"w1l_{l}"] = (w1 - w1h.astype(np.float32)).astype(BF16)
        shared[f"w2h_{l}"] = w2h
        shared[f"w2l_{l}"] = (w2 - w2h.astype(np.float32)).astype(BF16)
        shared[f"b1_{l}"] = np.asarray(
            inputs[f"b1_{l}"], dtype=np.float32).reshape(DH, 1)
        shared[f"b2_{l}"] = np.asarray(
            inputs[f"b2_{l}"], dtype=np.float32).reshape(DH, 1)

    in_maps = []
    for c in range(NCORES):
        m = dict(shared)
        m["a8"] = a8[c]
        in_maps.append(m)
    return in_maps


def get_program(inputs):
    """Build (or fetch cached) the bass program + per-core input maps."""
    cfg = _config(inputs)
    a8 = _prep_a8(inputs["edge_index"], cfg["N"], cfg["NPC"], cfg["MT"],
                  cfg["KC2"])
    key = (cfg["N"], cfg["DIN"], cfg["DH"], cfg["L"], NSPLIT)
    if key not in _BUILD_CACHE:
        _BUILD_CACHE[key] = _build(
            cfg["N"], cfg["DIN"], cfg["DH"], cfg["L"],
            cfg["NPC"], cfg["MT"], cfg["KC"], cfg["KC2"])
    nc = _BUILD_CACHE[key]
    in_maps = _make_in_maps(inputs, cfg, a8)
    return nc, in_maps, cfg


def kernel(**inputs):
    nc, in_maps, cfg = get_program(inputs)

    if os.environ.get("KERNEL_USE_SIM"):
        from concourse.bass_interp import MultiCoreSim
        sim = MultiCoreSim(nc, num_cores=NCORES)
        cores = list(sim.cores.values())
        for cid, cs in enumerate(cores):
            for name, val in in_maps[cid].items():
                cs.tensor(name)[:] = val
        sim.simulate(check_with_hw=False)
        parts = [np.asarray(cs.tensor("zout")) for cs in cores]
    else:
        from concourse import bass_utils
        res = bass_utils.run_bass_kernel_spmd(
            nc, in_maps, core_ids=list(range(NCORES)),
            trace=bool(os.environ.get("KERNEL_TRACE")),
        )
        kernel.last_results = res
        parts = [res.results[c]["zout"] for c in range(NCORES)]

    out = np.concatenate(parts, axis=0).astype(np.float32)
    return out


# revision 6
# speedup vs baseline: 1.6467x; 1.0914x over previous
"""Trainium2 Bass kernel for a 3-layer GIN encoder (gnn_message_passing).

Reference computation (per layer l):
    agg_i = sum_{j -> i} z_j          (scatter-add over edges)
    h     = z + agg                   (GIN eps=0, folded in as self-edges)
    z     = relu(relu(h @ w1 + b1) @ w2 + b2)

Distribution strategy (8 NeuronCores, SPMD single program):
  * Nodes block-sharded; edges partitioned by destination core so the
    aggregation is local; each layer's full activation table is AllGathered
    (the halo exchange for a dense random graph).  Internally nodes live in
    a padded index space (1280 slots/core, 30 dead) so every DMA and gather
    piece is 128-aligned; dead slots have zero adjacency everywhere.
  * Aggregation as a dense matmul with the local adjacency count matrix in
    fp8_e4m3 (counts are small ints -> exact).  The z table is also fp8
    (measured end-to-end rel err ~5e-3, bar is 2e-2), which enables
    MatmulPerfMode.DoubleRow: K=256 per instruction, 2x bf16 throughput.
  * The whole per-core adjacency (80 chunks x 1280 slots x 1B = 100KB per
    partition) stays resident in SBUF: streamed from HBM once during layer
    0, read for free in layers 1-2.
  * Each AllGather is split into one piece per MLP output group, launched
    as soon as that group's stores land; the next layer's aggregation
    consumes K-chunk pairs in piece-availability order so it starts as
    soon as the first piece arrives.
  * MLP in bf16 with hi/lo splits (3 product terms ~ fp32 accuracy),
    PSUM-accumulated; outputs transposed back via TensorE, stored fp8
    (f32 for the final layer).
"""

import os
import sys

sys.path.insert(0, "/opt/trn_rl_repo")

import numpy as np
import ml_dtypes

BF16 = ml_dtypes.bfloat16
FP8 = ml_dtypes.float8_e4m3  # TRN fp8e4 (max 240)
P = 128
NCORES = 8

# hi/lo product terms in the MLP matmuls (3 ~ fp32 accuracy)
NSPLIT = 3
# adjacency chunks fetched per stream DMA during layer 0
ABATCH = 4

_BUILD_CACHE: dict = {}


# --------------------------------------------------------------------------
# host-side preprocessing
# --------------------------------------------------------------------------

def _config(inputs):
    x = inputs["x"]
    N, DIN = int(x.shape[0]), int(x.shape[1])
    L = 0
    while f"w1_{L}" in inputs:
        L += 1
    DH = int(inputs["w1_0"].shape[1])
    assert N % NCORES == 0
    NPC = N // NCORES              # real rows per core (1250)
    MT = (NPC + P - 1) // P        # M-tiles per core (10)
    NPC2 = MT * P                  # padded rows per core (1280)
    N2 = NCORES * NPC2             # padded node space (10240)
    KC2 = N2 // P                  # zsb chunks (80, even)
    assert DIN % P == 0 and DH % P == 0 and MT % 2 == 0
    return dict(N=N, DIN=DIN, DH=DH, L=L, NPC=NPC, MT=MT, NPC2=NPC2,
                N2=N2, KC2=KC2)


def _prep_a8(edge_index, N, NPC, NPC2, KC2):
    """Dense transposed local adjacency per core, fp8, stream-batched.

    Src/dst in the padded index space.  Returns a8[c] of shape
    [KC2//ABATCH, P, ABATCH*NPC2] fp8 with
    a8[c][b, p, j*NPC2 + s] = #edges (src_pad = (ABATCH*b+j)*128+p) ->
    (dst = c*NPC2 + s), plus one self-edge per node.
    """
    src = np.asarray(edge_index[0], dtype=np.int64)
    dst = np.asarray(edge_index[1], dtype=np.int64)
    self_ix = np.arange(N, dtype=np.int64)
    allsrc = np.concatenate([src, self_ix])
    alldst = np.concatenate([dst, self_ix])
    # real -> padded index space
    allsrc = (allsrc // NPC) * NPC2 + allsrc % NPC
    gslot = (alldst // NPC) * NPC2 + alldst % NPC

    at = np.zeros((KC2 * P, NCORES * NPC2), np.float32)
    np.add.at(at, (allsrc, gslot), 1.0)
    at8 = at.astype(FP8)

    NB = KC2 // ABATCH
    a8 = []
    for c in range(NCORES):
        sl = at8[:, c * NPC2:(c + 1) * NPC2]
        a8.append(np.ascontiguousarray(
            sl.reshape(NB, ABATCH, P, NPC2)
              .transpose(0, 2, 1, 3)
              .reshape(NB, P, ABATCH * NPC2)))
    return a8


def _prep_xz(x, DIN, NPC, NPC2, KC2):
    """x in zsb layout over the padded space: xz[p, k*DIN+f] = x_pad[k*128+p, f]."""
    xf = np.zeros((KC2 * P, DIN), np.float32)
    xv = np.asarray(x, dtype=np.float32).reshape(NCORES, NPC, DIN)
    xf.reshape(NCORES, NPC2, DIN)[:, :NPC] = xv
    xz = xf.reshape(KC2, P, DIN).transpose(1, 0, 2).reshape(P, KC2 * DIN)
    return np.ascontiguousarray(xz).astype(FP8)


# --------------------------------------------------------------------------
# bass program
# --------------------------------------------------------------------------

def _build(N, DIN, DH, L, NPC, MT, NPC2, N2, KC2):
    from concourse import bacc, mybir, tile

    f32 = mybir.dt.float32
    bf = mybir.dt.bfloat16
    f8 = mybir.dt.float8e4
    SUB = mybir.AluOpType.subtract
    RELU = mybir.ActivationFunctionType.Relu
    DR = mybir.MatmulPerfMode.DoubleRow

    NKT2 = DH // P            # K/M tiles of the hidden dim (2)
    PAIRS = KC2 // 2
    NB = KC2 // ABATCH        # layer-0 stream batches
    # dst-slot groups; aligned with MLP M-tile groups of 4 (512 rows)
    NG = [(n0, min(512, NPC2 - n0)) for n0 in range(0, NPC2, 512)]
    GROUPS = [(g0, min(4, MT - g0)) for g0 in range(0, MT, 4)]

    # gather piece (per MLP group) that provides chunk k of the z table
    def chunk_piece(k):
        kl = k % MT
        for gi, (g0, gm) in enumerate(GROUPS):
            if kl < g0 + gm:
                return gi
        return len(GROUPS) - 1
    pair_order = sorted(range(PAIRS),
                        key=lambda p: (chunk_piece(2 * p), p))

    nc = bacc.Bacc(num_devices=NCORES)

    xzin = nc.dram_tensor("xz", [P, KC2 * DIN], f8, kind="ExternalInput")
    a8in = nc.dram_tensor("a8", [NB, P, ABATCH * NPC2], f8, kind="ExternalInput")
    identbin = nc.dram_tensor("identb", [P, P], bf, kind="ExternalInput")
    identfin = nc.dram_tensor("identf", [P, P], f32, kind="ExternalInput")
    win = {}
    for l in range(L):
        din = DIN if l == 0 else DH
        for nm, shp in [
            ("w1h", [din, DH]), ("w1l", [din, DH]),
            ("w2h", [DH, DH]), ("w2l", [DH, DH]),
        ]:
            win[(nm, l)] = nc.dram_tensor(f"{nm}_{l}", shp, bf, kind="ExternalInput")
        for nm in ("b1", "b2"):
            win[(nm, l)] = nc.dram_tensor(f"{nm}_{l}", [DH, 1], f32, kind="ExternalInput")
    zout = nc.dram_tensor("zout", [NPC, DH], f32, kind="ExternalOutput")

    with tile.TileContext(nc) as tc:
        with tc.tile_pool(name="const", bufs=1) as cp, \
             tc.tile_pool(name="zsbpool", bufs=1) as zsp, \
             tc.tile_pool(name="hpool", bufs=1) as hp, \
             tc.tile_pool(name="spool", bufs=2) as sp, \
             tc.tile_pool(name="zpool", bufs=1) as zp, \
             tc.tile_pool(name="zrpool", bufs=3) as zrp, \
             tc.tile_pool(name="hpsum", bufs=1, space="PSUM") as hpsum, \
             tc.tile_pool(name="mlppsum", bufs=2, space="PSUM") as mlppool, \
             tc.tile_pool(name="drampool", bufs=1, space="DRAM") as dp:

            # ---------------- resident constants ----------------
            identb_t = cp.tile([P, P], bf, name="identb_t")
            nc.gpsimd.dma_start(out=identb_t[:], in_=identbin[:, :])
            identf_t = cp.tile([P, P], f32, name="identf_t")
            nc.gpsimd.dma_start(out=identf_t[:], in_=identfin[:, :])

            wt = {}
            for l in range(L):
                din = DIN if l == 0 else DH
                nkt = din // P
                for nm, nk in (("w1h", nkt), ("w1l", nkt),
                               ("w2h", NKT2), ("w2l", NKT2)):
                    t = cp.tile([P, nk * DH], bf, name=f"{nm}{l}_t")
                    for kt in range(nk):
                        nc.gpsimd.dma_start(
                            out=t[:, kt * DH:(kt + 1) * DH],
                            in_=win[(nm, l)][kt * P:(kt + 1) * P, :])
                    wt[(nm, l)] = t
                for nm in ("b1", "b2"):
                    t = cp.tile([P, NKT2], f32, name=f"{nm}{l}_t")
                    for mo in range(NKT2):
                        nc.gpsimd.dma_start(
                            out=t[:, mo:mo + 1],
                            in_=win[(nm, l)][mo * P:(mo + 1) * P, :])
                    wt[(nm, l)] = t

            # resident adjacency: the whole per-core A.T in fp8
            acache = cp.tile([P, KC2, NPC2], f8, name="acache")

            # layer-boundary activation tables: one shared piece per
            # (layer, MLP group); piece gi holds rows [g0*P, (g0+gm)*P) of
            # every core's padded shard, concatenated by core
            zloc = [dp.tile([NPC2, DH], f8, name=f"zloc{l}")
                    for l in range(L - 1)]
            zfp = [[dp.tile([NCORES * gm * P, DH], f8,
                            name=f"zfp{l}_{gi}", addr_space="Shared")
                    for gi, (g0, gm) in enumerate(GROUPS)]
                   for l in range(L - 1)]

            # ---------------- layers ----------------
            for l in range(L):
                din = DIN if l == 0 else DH
                nkt = din // P
                last = (l == L - 1)

                # activation table -> SBUF, node-major chunks:
                # zsb[p, k, f] = z_pad[k*128+p, f]
                zsb = zsp.tile([P, KC2, din], f8, name=f"zsb_{l}", tag="zsb")
                if l == 0:
                    nc.scalar.dma_start(
                        out=zsb[:, :, :].rearrange("p k f -> p (k f)"),
                        in_=xzin[:, :])
                else:
                    # per (piece, core) aligned loads, piece-availability order
                    for gi, (g0, gm) in enumerate(GROUPS):
                        for q in range(NCORES):
                            k0 = q * MT + g0
                            nc.scalar.dma_start(
                                out=zsb[:, k0:k0 + gm, :],
                                in_=zfp[l - 1][gi]
                                    [q * gm * P:(q + 1) * gm * P, :]
                                    .rearrange("(k p) f -> p k f", p=P))

                # --- aggregation: h.T = z.T @ Aloc.T, fp8 DoubleRow pairs,
                # k-outer so each stationary zsb slice is loaded once
                hps = [hpsum.tile([P, nkt * 512], f32,
                                  name=f"hps{gi}_{l}", tag=f"hps{gi}")
                       for gi in range(len(NG))]
                porder = range(PAIRS) if l == 0 else pair_order
                for pi, p in enumerate(porder):
                    if l == 0 and p % (ABATCH // 2) == 0:
                        b = p // (ABATCH // 2)
                        nc.sync.dma_start(
                            out=acache[:, b * ABATCH:(b + 1) * ABATCH, :]
                                .rearrange("p k s -> p (k s)"),
                            in_=a8in[b, :, :])
                    for mf in range(nkt):
                        for gi, (n0, nn) in enumerate(NG):
                            nc.tensor.matmul(
                                out=hps[gi][:, mf * 512: mf * 512 + nn],
                                lhsT=zsb[:, 2 * p:2 * p + 2,
                                         mf * P:(mf + 1) * P],
                                rhs=acache[:, 2 * p:2 * p + 2, n0:n0 + nn],
                                start=(pi == 0),
                                stop=(pi == PAIRS - 1),
                                perf_mode=DR,
                            )

                # --- MLP per dst group (aligned with NG: 512 rows each)
                hhi = [hp.tile([P, NPC2], bf, name=f"hhi{mf}_{l}", tag=f"hhi{mf}")
                       for mf in range(nkt)]
                hlo = [hp.tile([P, NPC2], bf, name=f"hlo{mf}_{l}", tag=f"hlo{mf}")
                       for mf in range(nkt)]
                zT = [zp.tile([P, NPC2], f32 if last else bf,
                              name=f"zT{mo}_{l}",
                              tag=f"zT{mo}{'f' if last else 'b'}")
                      for mo in range(NKT2)]
                for gi, (g0, gm) in enumerate(GROUPS):
                    rows = gm * P
                    r0 = g0 * P
                    # split this group's h into hi/lo bf16
                    for mf in range(nkt):
                        nc.vector.tensor_copy(
                            out=hhi[mf][:, r0:r0 + rows],
                            in_=hps[gi][:, mf * 512: mf * 512 + rows])
                        nc.vector.tensor_tensor(
                            out=hlo[mf][:, r0:r0 + rows],
                            in0=hps[gi][:, mf * 512: mf * 512 + rows],
                            in1=hhi[mf][:, r0:r0 + rows],
                            op=SUB)

                    combos1 = [("w1h", hhi), ("w1h", hlo), ("w1l", hhi)][:NSPLIT]
                    s1h, s1l = [], []
                    for mo in range(NKT2):
                        p1 = mlppool.tile([P, 512], f32,
                                          name=f"p1_{l}_{g0}_{mo}", tag="mlp")
                        tot = len(combos1) * nkt
                        step = 0
                        for (wn, ht) in combos1:
                            for kt in range(nkt):
                                nc.tensor.matmul(
                                    out=p1[:, :rows],
                                    lhsT=wt[(wn, l)][:, kt * DH + mo * P: kt * DH + (mo + 1) * P],
                                    rhs=ht[kt][:, r0:r0 + rows],
                                    start=(step == 0), stop=(step == tot - 1))
                                step += 1
                        s1f = sp.tile([P, 512], f32, name=f"s1f_{l}_{g0}_{mo}", tag="s1f")
                        nc.scalar.activation(
                            out=s1f[:, :rows], in_=p1[:, :rows], func=RELU,
                            bias=wt[("b1", l)][:, mo:mo + 1])
                        sh = sp.tile([P, 512], bf, name=f"s1h_{l}_{g0}_{mo}", tag=f"s1h{mo}")
                        nc.vector.tensor_copy(out=sh[:, :rows], in_=s1f[:, :rows])
                        sl = sp.tile([P, 512], bf, name=f"s1l_{l}_{g0}_{mo}", tag=f"s1l{mo}")
                        nc.vector.tensor_tensor(
                            out=sl[:, :rows], in0=s1f[:, :rows], in1=sh[:, :rows], op=SUB)
                        s1h.append(sh)
                        s1l.append(sl)
                    combos2 = [("w2h", s1h), ("w2h", s1l), ("w2l", s1h)][:NSPLIT]
                    for mo in range(NKT2):
                        p2 = mlppool.tile([P, 512], f32,
                                          name=f"p2_{l}_{g0}_{mo}", tag="mlp")
                        tot = len(combos2) * NKT2
                        step = 0
                        for (wn, st) in combos2:
                            for kt in range(NKT2):
                                nc.tensor.matmul(
                                    out=p2[:, :rows],
                                    lhsT=wt[(wn, l)][:, kt * DH + mo * P: kt * DH + (mo + 1) * P],
                                    rhs=st[kt][:, :rows],
                                    start=(step == 0), stop=(step == tot - 1))
                                step += 1
                        nc.scalar.activation(
                            out=zT[mo][:, r0:r0 + rows], in_=p2[:, :rows], func=RELU,
                            bias=wt[("b2", l)][:, mo:mo + 1])

                    # transpose back to row-major + store
                    ident = identf_t if last else identb_t
                    for m in range(g0, g0 + gm):
                        rows_m = min(P, NPC - m * P) if last else P
                        tp = mlppool.tile([P, NKT2 * P], f32 if last else bf,
                                          name=f"tp_{l}_{m}", tag="mlp")
                        for mo in range(NKT2):
                            nc.tensor.transpose(
                                out=tp[:, mo * P:(mo + 1) * P],
                                in_=zT[mo][:, m * P:(m + 1) * P],
                                identity=ident[:])
                        zr = zrp.tile([P, NKT2 * P], f32 if last else f8,
                                      name=f"zr_{l}_{m}", tag="zr")
                        nc.vector.tensor_copy(out=zr[:], in_=tp[:])
                        dst = zout if last else zloc[l]
                        nc.sync.dma_start(
                            out=dst[m * P: m * P + rows_m, :],
                            in_=zr[:rows_m, :])

                    # launch this group's gather piece as soon as its
                    # stores land; the next layer starts on piece 0
                    if not last:
                        nc.gpsimd.collective_compute(
                            "AllGather",
                            mybir.AluOpType.bypass,
                            replica_groups=[list(range(NCORES))],
                            ins=[zloc[l][r0:r0 + rows, :].opt()],
                            outs=[zfp[l][gi][:, :].opt()],
                        )

    nc.compile()
    return nc


# --------------------------------------------------------------------------
# entry point
# --------------------------------------------------------------------------

def _make_in_maps(inputs, cfg, a8):
    DIN, DH, L = cfg["DIN"], cfg["DH"], cfg["L"]
    xz = _prep_xz(inputs["x"], DIN, cfg["NPC"], cfg["NPC2"], cfg["KC2"])
    identb = np.eye(P, dtype=np.float32).astype(BF16)
    identf = np.eye(P, dtype=np.float32)

    shared = {"xz": xz, "identb": identb, "identf": identf}
    for l in range(L):
        w1 = np.asarray(inputs[f"w1_{l}"], dtype=np.float32)
        w2 = np.asarray(inputs[f"w2_{l}"], dtype=np.float32)
        w1h = w1.astype(BF16)
        w2h = w2.astype(BF16)
        shared[f"w1h_{l}"] = w1h
        shared[f"w1l_{l}"] = (w1 - w1h.astype(np.float32)).astype(BF16)
        shared[f"w2h_{l}"] = w2h
        shared[f"w2l_{l}"] = (w2 - w2h.astype(np.float32)).astype(BF16)
        shared[f"b1_{l}"] = np.asarray(
            inputs[f"b1_{l}"], dtype=np.float32).reshape(DH, 1)
        shared[f"b2_{l}"] = np.asarray(
            inputs[f"b2_{l}"], dtype=np.float32).reshape(DH, 1)

    in_maps = []
    for c in range(NCORES):
        m = dict(shared)
        m["a8"] = a8[c]
        in_maps.append(m)
    return in_maps


def get_program(inputs):
    """Build (or fetch cached) the bass program + per-core input maps."""
    cfg = _config(inputs)
    a8 = _prep_a8(inputs["edge_index"], cfg["N"], cfg["NPC"], cfg["NPC2"],
                  cfg["KC2"])
    key = (cfg["N"], cfg["DIN"], cfg["DH"], cfg["L"], NSPLIT)
    if key not in _BUILD_CACHE:
        _BUILD_CACHE[key] = _build(
            cfg["N"], cfg["DIN"], cfg["DH"], cfg["L"],
            cfg["NPC"], cfg["MT"], cfg["NPC2"], cfg["N2"], cfg["KC2"])
    nc = _BUILD_CACHE[key]
    in_maps = _make_in_maps(inputs, cfg, a8)
    return nc, in_maps, cfg


def kernel(**inputs):
    nc, in_maps, cfg = get_program(inputs)

    if os.environ.get("KERNEL_USE_SIM"):
        from concourse.bass_interp import MultiCoreSim
        sim = MultiCoreSim(nc, num_cores=NCORES)
        cores = list(sim.cores.values())
        for cid, cs in enumerate(cores):
            for name, val in in_maps[cid].items():
                cs.tensor(name)[:] = val
        sim.simulate(check_with_hw=False)
        parts = [np.asarray(cs.tensor("zout")) for cs in cores]
    else:
        from concourse import bass_utils
        res = bass_utils.run_bass_kernel_spmd(
            nc, in_maps, core_ids=list(range(NCORES)),
            trace=bool(os.environ.get("KERNEL_TRACE")),
        )
        kernel.last_results = res
        parts = [res.results[c]["zout"] for c in range(NCORES)]

    out = np.concatenate(parts, axis=0).astype(np.float32)
    return out


# revision 9
# speedup vs baseline: 1.7429x; 1.0584x over previous
"""Trainium2 Bass kernel for a 3-layer GIN encoder (gnn_message_passing).

Reference computation (per layer l):
    agg_i = sum_{j -> i} z_j          (scatter-add over edges)
    h     = z + agg                   (GIN eps=0, folded in as self-edges)
    z     = relu(relu(h @ w1 + b1) @ w2 + b2)

Distribution strategy (8 NeuronCores, SPMD single program):
  * Nodes block-sharded; edges partitioned by destination core so the
    aggregation is local; each layer's full activation table is AllGathered
    (the halo exchange for a dense random graph).  Internally nodes live in
    a padded index space (1280 slots/core, 30 dead) so every DMA and gather
    piece is 128-aligned; dead slots have zero adjacency everywhere.
  * Aggregation as a dense matmul with the local adjacency count matrix in
    fp8_e4m3 (counts are small ints -> exact).  The z table is also fp8
    (measured end-to-end rel err ~5e-3, bar is 2e-2), which enables
    MatmulPerfMode.DoubleRow: K=256 per instruction, 2x bf16 throughput.
  * The whole per-core adjacency (80 chunks x 1280 slots x 1B = 100KB per
    partition) stays resident in SBUF: streamed from HBM once during layer
    0, read for free in layers 1-2.
  * Each AllGather is split into one piece per MLP output group, launched
    as soon as that group's stores land; the next layer's aggregation
    consumes K-chunk pairs in piece-availability order so it starts as
    soon as the first piece arrives.
  * MLP in bf16 with hi/lo splits (3 product terms ~ fp32 accuracy),
    PSUM-accumulated; outputs transposed back via TensorE, stored fp8
    (f32 for the final layer).
"""

import os
import sys

sys.path.insert(0, "/opt/trn_rl_repo")

import numpy as np
import ml_dtypes

BF16 = ml_dtypes.bfloat16
FP8 = ml_dtypes.float8_e4m3  # TRN fp8e4 (max 240)
P = 128
NCORES = 8

# hi/lo product terms in the MLP matmuls (3 ~ fp32 accuracy)
NSPLIT = 3
# adjacency chunks fetched per stream DMA during layer 0
ABATCH = 4

_BUILD_CACHE: dict = {}


# --------------------------------------------------------------------------
# host-side preprocessing
# --------------------------------------------------------------------------

def _config(inputs):
    x = inputs["x"]
    N, DIN = int(x.shape[0]), int(x.shape[1])
    L = 0
    while f"w1_{L}" in inputs:
        L += 1
    DH = int(inputs["w1_0"].shape[1])
    assert N % NCORES == 0
    NPC = N // NCORES              # real rows per core (1250)
    MT = (NPC + P - 1) // P        # M-tiles per core (10)
    NPC2 = MT * P                  # padded rows per core (1280)
    N2 = NCORES * NPC2             # padded node space (10240)
    KC2 = N2 // P                  # zsb chunks (80, even)
    assert DIN % P == 0 and DH % P == 0 and MT % 2 == 0
    return dict(N=N, DIN=DIN, DH=DH, L=L, NPC=NPC, MT=MT, NPC2=NPC2,
                N2=N2, KC2=KC2)


def _prep_a8(edge_index, N, NPC, NPC2, KC2):
    """Dense transposed local adjacency per core, fp8, stream-batched.

    Src/dst in the padded index space.  Returns a8[c] of shape
    [KC2//ABATCH, P, ABATCH*NPC2] fp8 with
    a8[c][b, p, j*NPC2 + s] = #edges (src_pad = (ABATCH*b+j)*128+p) ->
    (dst = c*NPC2 + s), plus one self-edge per node.
    """
    src = np.asarray(edge_index[0], dtype=np.int64)
    dst = np.asarray(edge_index[1], dtype=np.int64)
    self_ix = np.arange(N, dtype=np.int64)
    allsrc = np.concatenate([src, self_ix])
    alldst = np.concatenate([dst, self_ix])
    # real -> padded index space
    allsrc = (allsrc // NPC) * NPC2 + allsrc % NPC
    gslot = (alldst // NPC) * NPC2 + alldst % NPC

    at = np.zeros((KC2 * P, NCORES * NPC2), np.float32)
    np.add.at(at, (allsrc, gslot), 1.0)
    at8 = at.astype(FP8)

    NB = KC2 // ABATCH
    a8 = []
    for c in range(NCORES):
        sl = at8[:, c * NPC2:(c + 1) * NPC2]
        a8.append(np.ascontiguousarray(
            sl.reshape(NB, ABATCH, P, NPC2)
              .transpose(0, 2, 1, 3)
              .reshape(NB, P, ABATCH * NPC2)))
    return a8


def _prep_xz(x, DIN, NPC, NPC2, KC2):
    """x in zsb layout over the padded space: xz[p, k*DIN+f] = x_pad[k*128+p, f]."""
    xf = np.zeros((KC2 * P, DIN), np.float32)
    xv = np.asarray(x, dtype=np.float32).reshape(NCORES, NPC, DIN)
    xf.reshape(NCORES, NPC2, DIN)[:, :NPC] = xv
    xz = xf.reshape(KC2, P, DIN).transpose(1, 0, 2).reshape(P, KC2 * DIN)
    return np.ascontiguousarray(xz).astype(FP8)


# --------------------------------------------------------------------------
# bass program
# --------------------------------------------------------------------------

def _build(N, DIN, DH, L, NPC, MT, NPC2, N2, KC2):
    from concourse import bacc, mybir, tile

    f32 = mybir.dt.float32
    bf = mybir.dt.bfloat16
    f8 = mybir.dt.float8e4
    SUB = mybir.AluOpType.subtract
    RELU = mybir.ActivationFunctionType.Relu
    DR = mybir.MatmulPerfMode.DoubleRow

    NKT2 = DH // P            # K/M tiles of the hidden dim (2)
    PAIRS = KC2 // 2
    NB = KC2 // ABATCH        # layer-0 stream batches
    # dst-slot groups; aligned with MLP M-tile groups of 4 (512 rows)
    NG = [(n0, min(512, NPC2 - n0)) for n0 in range(0, NPC2, 512)]
    GROUPS = [(g0, min(4, MT - g0)) for g0 in range(0, MT, 4)]

    # gather piece (per MLP group) that provides chunk k of the z table
    def chunk_piece(k):
        kl = k % MT
        for gi, (g0, gm) in enumerate(GROUPS):
            if kl < g0 + gm:
                return gi
        return len(GROUPS) - 1
    pair_order = sorted(range(PAIRS),
                        key=lambda p: (chunk_piece(2 * p), p))

    nc = bacc.Bacc(num_devices=NCORES)

    xzin = nc.dram_tensor("xz", [P, KC2 * DIN], f8, kind="ExternalInput")
    a8in = nc.dram_tensor("a8", [NB, P, ABATCH * NPC2], f8, kind="ExternalInput")
    identbin = nc.dram_tensor("identb", [P, P], bf, kind="ExternalInput")
    identfin = nc.dram_tensor("identf", [P, P], f32, kind="ExternalInput")
    win = {}
    for l in range(L):
        din = DIN if l == 0 else DH
        for nm, shp in [
            ("w1h", [din, DH]), ("w1l", [din, DH]),
            ("w2h", [DH, DH]), ("w2l", [DH, DH]),
        ]:
            win[(nm, l)] = nc.dram_tensor(f"{nm}_{l}", shp, bf, kind="ExternalInput")
        for nm in ("b1", "b2"):
            win[(nm, l)] = nc.dram_tensor(f"{nm}_{l}", [DH, 1], f32, kind="ExternalInput")
    zout = nc.dram_tensor("zout", [NPC, DH], f32, kind="ExternalOutput")

    with tile.TileContext(nc) as tc:
        with tc.tile_pool(name="const", bufs=1) as cp, \
             tc.tile_pool(name="zsbpool", bufs=1) as zsp, \
             tc.tile_pool(name="hpool", bufs=1) as hp, \
             tc.tile_pool(name="spool", bufs=2) as sp, \
             tc.tile_pool(name="zpool", bufs=1) as zp, \
             tc.tile_pool(name="zrpool", bufs=3) as zrp, \
             tc.tile_pool(name="hpsum", bufs=1, space="PSUM") as hpsum, \
             tc.tile_pool(name="mlppsum", bufs=2, space="PSUM") as mlppool, \
             tc.tile_pool(name="drampool", bufs=1, space="DRAM") as dp:

            # ---------------- resident constants ----------------
            identb_t = cp.tile([P, P], bf, name="identb_t")
            nc.gpsimd.dma_start(out=identb_t[:], in_=identbin[:, :])
            identf_t = cp.tile([P, P], f32, name="identf_t")
            nc.gpsimd.dma_start(out=identf_t[:], in_=identfin[:, :])

            wt = {}
            for l in range(L):
                din = DIN if l == 0 else DH
                nkt = din // P
                for nm, nk in (("w1h", nkt), ("w1l", nkt),
                               ("w2h", NKT2), ("w2l", NKT2)):
                    t = cp.tile([P, nk * DH], bf, name=f"{nm}{l}_t")
                    for kt in range(nk):
                        nc.gpsimd.dma_start(
                            out=t[:, kt * DH:(kt + 1) * DH],
                            in_=win[(nm, l)][kt * P:(kt + 1) * P, :])
                    wt[(nm, l)] = t
                for nm in ("b1", "b2"):
                    t = cp.tile([P, NKT2], f32, name=f"{nm}{l}_t")
                    for mo in range(NKT2):
                        nc.gpsimd.dma_start(
                            out=t[:, mo:mo + 1],
                            in_=win[(nm, l)][mo * P:(mo + 1) * P, :])
                    wt[(nm, l)] = t

            # resident adjacency: the whole per-core A.T in fp8
            acache = cp.tile([P, KC2, NPC2], f8, name="acache")

            # layer-boundary activation tables: one shared piece per
            # (layer, MLP group); piece gi holds rows [g0*P, (g0+gm)*P) of
            # every core's padded shard, concatenated by core
            zloc = [dp.tile([NPC2, DH], f8, name=f"zloc{l}")
                    for l in range(L - 1)]
            zfp = [[dp.tile([NCORES * gm * P, DH], f8,
                            name=f"zfp{l}_{gi}", addr_space="Shared")
                    for gi, (g0, gm) in enumerate(GROUPS)]
                   for l in range(L - 1)]

            # ---------------- layers ----------------
            for l in range(L):
                din = DIN if l == 0 else DH
                nkt = din // P
                last = (l == L - 1)

                # activation table -> SBUF, node-major chunks:
                # zsb[p, k, f] = z_pad[k*128+p, f]
                zsb = zsp.tile([P, KC2, din], f8, name=f"zsb_{l}", tag="zsb")
                if l == 0:
                    nc.scalar.dma_start(
                        out=zsb[:, :, :].rearrange("p k f -> p (k f)"),
                        in_=xzin[:, :])
                else:
                    # per (piece, core) aligned loads, piece-availability
                    # order, alternating issue queues
                    for gi, (g0, gm) in enumerate(GROUPS):
                        for q in range(NCORES):
                            k0 = q * MT + g0
                            eng = nc.scalar if q % 2 == 0 else nc.sync
                            eng.dma_start(
                                out=zsb[:, k0:k0 + gm, :],
                                in_=zfp[l - 1][gi]
                                    [q * gm * P:(q + 1) * gm * P, :]
                                    .rearrange("(k p) f -> p k f", p=P))

                # --- aggregation: h.T = z.T @ Aloc.T, fp8 DoubleRow pairs,
                # k-outer so each stationary zsb slice is loaded once
                hps = [hpsum.tile([P, nkt * 512], f32,
                                  name=f"hps{gi}_{l}", tag=f"hps{gi}")
                       for gi in range(len(NG))]
                porder = list(range(PAIRS)) if l == 0 else pair_order

                def agg_phase(gis, stream):
                    for pi, p in enumerate(porder):
                        if stream and p % (ABATCH // 2) == 0:
                            b = p // (ABATCH // 2)
                            nc.sync.dma_start(
                                out=acache[:, b * ABATCH:(b + 1) * ABATCH, :]
                                    .rearrange("p k s -> p (k s)"),
                                in_=a8in[b, :, :])
                        for mf in range(nkt):
                            for gi in gis:
                                n0, nn = NG[gi]
                                nc.tensor.matmul(
                                    out=hps[gi][:, mf * 512: mf * 512 + nn],
                                    lhsT=zsb[:, 2 * p:2 * p + 2,
                                             mf * P:(mf + 1) * P],
                                    rhs=acache[:, 2 * p:2 * p + 2, n0:n0 + nn],
                                    start=(pi == 0),
                                    stop=(pi == PAIRS - 1),
                                    perf_mode=DR,
                                )

                agg_phase([0], stream=(l == 0))

                # --- MLP per dst group (aligned with NG: 512 rows each)
                hhi = [hp.tile([P, NPC2], bf, name=f"hhi{mf}_{l}", tag=f"hhi{mf}")
                       for mf in range(nkt)]
                hlo = [hp.tile([P, NPC2], bf, name=f"hlo{mf}_{l}", tag=f"hlo{mf}")
                       for mf in range(nkt)]
                zT = [zp.tile([P, NPC2], f32 if last else bf,
                              name=f"zT{mo}_{l}",
                              tag=f"zT{mo}{'f' if last else 'b'}")
                      for mo in range(NKT2)]
                def mlp_group(gi):
                    g0, gm = GROUPS[gi]
                    rows = gm * P
                    r0 = g0 * P
                    # split this group's h into hi/lo bf16
                    for mf in range(nkt):
                        nc.vector.tensor_copy(
                            out=hhi[mf][:, r0:r0 + rows],
                            in_=hps[gi][:, mf * 512: mf * 512 + rows])
                        nc.vector.tensor_tensor(
                            out=hlo[mf][:, r0:r0 + rows],
                            in0=hps[gi][:, mf * 512: mf * 512 + rows],
                            in1=hhi[mf][:, r0:r0 + rows],
                            op=SUB)

                    combos1 = [("w1h", hhi), ("w1h", hlo), ("w1l", hhi)][:NSPLIT]
                    s1h, s1l = [], []
                    for mo in range(NKT2):
                        p1 = mlppool.tile([P, 512], f32,
                                          name=f"p1_{l}_{g0}_{mo}", tag="mlp")
                        tot = len(combos1) * nkt
                        step = 0
                        for (wn, ht) in combos1:
                            for kt in range(nkt):
                                nc.tensor.matmul(
                                    out=p1[:, :rows],
                                    lhsT=wt[(wn, l)][:, kt * DH + mo * P: kt * DH + (mo + 1) * P],
                                    rhs=ht[kt][:, r0:r0 + rows],
                                    start=(step == 0), stop=(step == tot - 1))
                                step += 1
                        s1f = sp.tile([P, 512], f32, name=f"s1f_{l}_{g0}_{mo}", tag="s1f")
                        nc.scalar.activation(
                            out=s1f[:, :rows], in_=p1[:, :rows], func=RELU,
                            bias=wt[("b1", l)][:, mo:mo + 1])
                        sh = sp.tile([P, 512], bf, name=f"s1h_{l}_{g0}_{mo}", tag=f"s1h{mo}")
                        nc.vector.tensor_copy(out=sh[:, :rows], in_=s1f[:, :rows])
                        sl = sp.tile([P, 512], bf, name=f"s1l_{l}_{g0}_{mo}", tag=f"s1l{mo}")
                        nc.vector.tensor_tensor(
                            out=sl[:, :rows], in0=s1f[:, :rows], in1=sh[:, :rows], op=SUB)
                        s1h.append(sh)
                        s1l.append(sl)
                    combos2 = [("w2h", s1h), ("w2h", s1l), ("w2l", s1h)][:NSPLIT]
                    for mo in range(NKT2):
                        p2 = mlppool.tile([P, 512], f32,
                                          name=f"p2_{l}_{g0}_{mo}", tag="mlp")
                        tot = len(combos2) * NKT2
                        step = 0
                        for (wn, st) in combos2:
                            for kt in range(NKT2):
                                nc.tensor.matmul(
                                    out=p2[:, :rows],
                                    lhsT=wt[(wn, l)][:, kt * DH + mo * P: kt * DH + (mo + 1) * P],
                                    rhs=st[kt][:, :rows],
                                    start=(step == 0), stop=(step == tot - 1))
                                step += 1
                        nc.scalar.activation(
                            out=zT[mo][:, r0:r0 + rows], in_=p2[:, :rows], func=RELU,
                            bias=wt[("b2", l)][:, mo:mo + 1])

                    # transpose back to row-major + store
                    ident = identf_t if last else identb_t
                    for m in range(g0, g0 + gm):
                        rows_m = min(P, NPC - m * P) if last else P
                        tp = mlppool.tile([P, NKT2 * P], f32 if last else bf,
                                          name=f"tp_{l}_{m}", tag="mlp")
                        for mo in range(NKT2):
                            nc.tensor.transpose(
                                out=tp[:, mo * P:(mo + 1) * P],
                                in_=zT[mo][:, m * P:(m + 1) * P],
                                identity=ident[:])
                        zr = zrp.tile([P, NKT2 * P], f32 if last else f8,
                                      name=f"zr_{l}_{m}", tag="zr")
                        nc.vector.tensor_copy(out=zr[:], in_=tp[:])
                        dst = zout if last else zloc[l]
                        nc.sync.dma_start(
                            out=dst[m * P: m * P + rows_m, :],
                            in_=zr[:rows_m, :])

                    # launch this group's gather piece as soon as its
                    # stores land; the next layer starts on piece 0
                    if not last:
                        nc.gpsimd.collective_compute(
                            "AllGather",
                            mybir.AluOpType.bypass,
                            replica_groups=[list(range(NCORES))],
                            ins=[zloc[l][r0:r0 + rows, :].opt()],
                            outs=[zfp[l][gi][:, :].opt()],
                        )

                mlp_group(0)
                agg_phase([1, 2], stream=False)
                mlp_group(1)
                mlp_group(2)

    nc.compile()
    return nc


# --------------------------------------------------------------------------
# entry point
# --------------------------------------------------------------------------

def _make_in_maps(inputs, cfg, a8):
    DIN, DH, L = cfg["DIN"], cfg["DH"], cfg["L"]
    xz = _prep_xz(inputs["x"], DIN, cfg["NPC"], cfg["NPC2"], cfg["KC2"])
    identb = np.eye(P, dtype=np.float32).astype(BF16)
    identf = np.eye(P, dtype=np.float32)

    shared = {"xz": xz, "identb": identb, "identf": identf}
    for l in range(L):
        w1 = np.asarray(inputs[f"w1_{l}"], dtype=np.float32)
        w2 = np.asarray(inputs[f"w2_{l}"], dtype=np.float32)
        w1h = w1.astype(BF16)
        w2h = w2.astype(BF16)
        shared[f"w1h_{l}"] = w1h
        shared[f"w1l_{l}"] = (w1 - w1h.astype(np.float32)).astype(BF16)
        shared[f"w2h_{l}"] = w2h
        shared[f"w2l_{l}"] = (w2 - w2h.astype(np.float32)).astype(BF16)
        shared[f"b1_{l}"] = np.asarray(
            inputs[f"b1_{l}"], dtype=np.float32).reshape(DH, 1)
        shared[f"b2_{l}"] = np.asarray(
            inputs[f"b2_{l}"], dtype=np.float32).reshape(DH, 1)

    in_maps = []
    for c in range(NCORES):
        m = dict(shared)
        m["a8"] = a8[c]
        in_maps.append(m)
    return in_maps


def get_program(inputs):
    """Build (or fetch cached) the bass program + per-core input maps."""
    cfg = _config(inputs)
    a8 = _prep_a8(inputs["edge_index"], cfg["N"], cfg["NPC"], cfg["NPC2"],
                  cfg["KC2"])
    key = (cfg["N"], cfg["DIN"], cfg["DH"], cfg["L"], NSPLIT)
    if key not in _BUILD_CACHE:
        _BUILD_CACHE[key] = _build(
            cfg["N"], cfg["DIN"], cfg["DH"], cfg["L"],
            cfg["NPC"], cfg["MT"], cfg["NPC2"], cfg["N2"], cfg["KC2"])
    nc = _BUILD_CACHE[key]
    in_maps = _make_in_maps(inputs, cfg, a8)
    return nc, in_maps, cfg


def kernel(**inputs):
    nc, in_maps, cfg = get_program(inputs)

    if os.environ.get("KERNEL_USE_SIM"):
        from concourse.bass_interp import MultiCoreSim
        sim = MultiCoreSim(nc, num_cores=NCORES)
        cores = list(sim.cores.values())
        for cid, cs in enumerate(cores):
            for name, val in in_maps[cid].items():
                cs.tensor(name)[:] = val
        sim.simulate(check_with_hw=False)
        parts = [np.asarray(cs.tensor("zout")) for cs in cores]
    else:
        from concourse import bass_utils
        res = bass_utils.run_bass_kernel_spmd(
            nc, in_maps, core_ids=list(range(NCORES)),
            trace=bool(os.environ.get("KERNEL_TRACE")),
        )
        kernel.last_results = res
        parts = [res.results[c]["zout"] for c in range(NCORES)]

    out = np.concatenate(parts, axis=0).astype(np.float32)
    return out


# revision 12
# speedup vs baseline: 1.7548x; 1.0068x over previous
"""Trainium2 Bass kernel for a 3-layer GIN encoder (gnn_message_passing).

Reference computation (per layer l):
    agg_i = sum_{j -> i} z_j          (scatter-add over edges)
    h     = z + agg                   (GIN eps=0, folded in as self-edges)
    z     = relu(relu(h @ w1 + b1) @ w2 + b2)

Distribution strategy (8 NeuronCores, SPMD single program):
  * Nodes block-sharded; edges partitioned by destination core so the
    aggregation is local; each layer's full activation table is AllGathered
    (the halo exchange for a dense random graph).  Internally nodes live in
    a padded index space (1280 slots/core, 30 dead) so every DMA and gather
    piece is 128-aligned; dead slots have zero adjacency everywhere.
  * Aggregation as a dense matmul with the local adjacency count matrix in
    fp8_e4m3 (counts are small ints -> exact).  The z table is also fp8
    (measured end-to-end rel err ~5e-3, bar is 2e-2), which enables
    MatmulPerfMode.DoubleRow: K=256 per instruction, 2x bf16 throughput.
  * The whole per-core adjacency (80 chunks x 1280 slots x 1B = 100KB per
    partition) stays resident in SBUF: streamed from HBM once during layer
    0, read for free in layers 1-2.
  * Each AllGather is split into one piece per MLP output group, launched
    as soon as that group's stores land; the next layer's aggregation
    consumes K-chunk pairs in piece-availability order so it starts as
    soon as the first piece arrives.
  * MLP in bf16 with hi/lo splits (3 product terms ~ fp32 accuracy),
    PSUM-accumulated; outputs transposed back via TensorE, stored fp8
    (f32 for the final layer).
"""

import os
import sys

sys.path.insert(0, "/opt/trn_rl_repo")

import numpy as np
import ml_dtypes

BF16 = ml_dtypes.bfloat16
FP8 = ml_dtypes.float8_e4m3  # TRN fp8e4 (max 240)
P = 128
NCORES = 8

# hi/lo product terms in the MLP matmuls (3 ~ fp32 accuracy)
NSPLIT = 3
# adjacency chunks fetched per stream DMA during layer 0
ABATCH = 4

_BUILD_CACHE: dict = {}


# --------------------------------------------------------------------------
# host-side preprocessing
# --------------------------------------------------------------------------

def _config(inputs):
    x = inputs["x"]
    N, DIN = int(x.shape[0]), int(x.shape[1])
    L = 0
    while f"w1_{L}" in inputs:
        L += 1
    DH = int(inputs["w1_0"].shape[1])
    assert N % NCORES == 0
    NPC = N // NCORES              # real rows per core (1250)
    MT = (NPC + P - 1) // P        # M-tiles per core (10)
    NPC2 = MT * P                  # padded rows per core (1280)
    N2 = NCORES * NPC2             # padded node space (10240)
    KC2 = N2 // P                  # zsb chunks (80, even)
    assert DIN % P == 0 and DH % P == 0 and MT % 2 == 0
    return dict(N=N, DIN=DIN, DH=DH, L=L, NPC=NPC, MT=MT, NPC2=NPC2,
                N2=N2, KC2=KC2)


def _prep_a8(edge_index, N, NPC, NPC2, KC2):
    """Dense transposed local adjacency per core, fp8, stream-batched.

    Src/dst in the padded index space.  Returns a8[c] of shape
    [KC2//ABATCH, P, ABATCH*NPC2] fp8 with
    a8[c][b, p, j*NPC2 + s] = #edges (src_pad = (ABATCH*b+j)*128+p) ->
    (dst = c*NPC2 + s), plus one self-edge per node.
    """
    src = np.asarray(edge_index[0], dtype=np.int64)
    dst = np.asarray(edge_index[1], dtype=np.int64)
    self_ix = np.arange(N, dtype=np.int64)
    allsrc = np.concatenate([src, self_ix])
    alldst = np.concatenate([dst, self_ix])
    # real -> padded index space
    allsrc = (allsrc // NPC) * NPC2 + allsrc % NPC
    gslot = (alldst // NPC) * NPC2 + alldst % NPC

    at = np.zeros((KC2 * P, NCORES * NPC2), np.float32)
    np.add.at(at, (allsrc, gslot), 1.0)
    at8 = at.astype(FP8)

    NB = KC2 // ABATCH
    a8 = []
    for c in range(NCORES):
        sl = at8[:, c * NPC2:(c + 1) * NPC2]
        a8.append(np.ascontiguousarray(
            sl.reshape(NB, ABATCH, P, NPC2)
              .transpose(0, 2, 1, 3)
              .reshape(NB, P, ABATCH * NPC2)))
    return a8


def _prep_xz(x, DIN, NPC, NPC2, KC2):
    """x in zsb layout over the padded space: xz[p, k*DIN+f] = x_pad[k*128+p, f]."""
    xf = np.zeros((KC2 * P, DIN), np.float32)
    xv = np.asarray(x, dtype=np.float32).reshape(NCORES, NPC, DIN)
    xf.reshape(NCORES, NPC2, DIN)[:, :NPC] = xv
    xz = xf.reshape(KC2, P, DIN).transpose(1, 0, 2).reshape(P, KC2 * DIN)
    return np.ascontiguousarray(xz).astype(FP8)


# --------------------------------------------------------------------------
# bass program
# --------------------------------------------------------------------------

def _build(N, DIN, DH, L, NPC, MT, NPC2, N2, KC2):
    from concourse import bacc, mybir, tile

    f32 = mybir.dt.float32
    bf = mybir.dt.bfloat16
    f8 = mybir.dt.float8e4
    SUB = mybir.AluOpType.subtract
    RELU = mybir.ActivationFunctionType.Relu
    DR = mybir.MatmulPerfMode.DoubleRow

    NKT2 = DH // P            # K/M tiles of the hidden dim (2)
    PAIRS = KC2 // 2
    NB = KC2 // ABATCH        # layer-0 stream batches
    # dst-slot groups; aligned with MLP M-tile groups of 4 (512 rows)
    NG = [(n0, min(512, NPC2 - n0)) for n0 in range(0, NPC2, 512)]
    GROUPS = [(g0, min(4, MT - g0)) for g0 in range(0, MT, 4)]

    # gather piece (per MLP group) that provides chunk k of the z table
    def chunk_piece(k):
        kl = k % MT
        for gi, (g0, gm) in enumerate(GROUPS):
            if kl < g0 + gm:
                return gi
        return len(GROUPS) - 1
    pair_order = sorted(range(PAIRS),
                        key=lambda p: (chunk_piece(2 * p), p))

    nc = bacc.Bacc(num_devices=NCORES)

    xzin = nc.dram_tensor("xz", [P, KC2 * DIN], f8, kind="ExternalInput")
    a8in = nc.dram_tensor("a8", [NB, P, ABATCH * NPC2], f8, kind="ExternalInput")
    identbin = nc.dram_tensor("identb", [P, P], bf, kind="ExternalInput")
    identfin = nc.dram_tensor("identf", [P, P], f32, kind="ExternalInput")
    win = {}
    for l in range(L):
        din = DIN if l == 0 else DH
        for nm, shp in [
            ("w1h", [din, DH]), ("w1l", [din, DH]),
            ("w2h", [DH, DH]), ("w2l", [DH, DH]),
        ]:
            win[(nm, l)] = nc.dram_tensor(f"{nm}_{l}", shp, bf, kind="ExternalInput")
        for nm in ("b1", "b2"):
            win[(nm, l)] = nc.dram_tensor(f"{nm}_{l}", [DH, 1], f32, kind="ExternalInput")
    zout = nc.dram_tensor("zout", [NPC, DH], f32, kind="ExternalOutput")

    with tile.TileContext(nc) as tc:
        with tc.tile_pool(name="const", bufs=1) as cp, \
             tc.tile_pool(name="zsbpool", bufs=1) as zsp, \
             tc.tile_pool(name="hpool", bufs=1) as hp, \
             tc.tile_pool(name="spool", bufs=2) as sp, \
             tc.tile_pool(name="zpool", bufs=1) as zp, \
             tc.tile_pool(name="zrpool", bufs=3) as zrp, \
             tc.tile_pool(name="hpsum", bufs=1, space="PSUM") as hpsum, \
             tc.tile_pool(name="mlppsum", bufs=2, space="PSUM") as mlppool, \
             tc.tile_pool(name="drampool", bufs=1, space="DRAM") as dp:

            # ---------------- resident constants ----------------
            identb_t = cp.tile([P, P], bf, name="identb_t")
            nc.gpsimd.dma_start(out=identb_t[:], in_=identbin[:, :])
            identf_t = cp.tile([P, P], f32, name="identf_t")
            nc.gpsimd.dma_start(out=identf_t[:], in_=identfin[:, :])

            ccsrc = dp.tile([P, 4], bf, name="ccsrc")
            nc.gpsimd.dma_start(out=ccsrc[:, :], in_=identb_t[:, 0:4])
            ccwarm = dp.tile([NCORES * P, 4], bf, name="ccwarm", addr_space="Shared")
            nc.gpsimd.collective_compute(
                "AllGather",
                mybir.AluOpType.bypass,
                replica_groups=[list(range(NCORES))],
                ins=[ccsrc[:, :].opt()],
                outs=[ccwarm[:, :].opt()],
            )

            wt = {}
            for l in range(L):
                din = DIN if l == 0 else DH
                nkt = din // P
                for nm, nk in (("w1h", nkt), ("w1l", nkt),
                               ("w2h", NKT2), ("w2l", NKT2)):
                    t = cp.tile([P, nk * DH], bf, name=f"{nm}{l}_t")
                    for kt in range(nk):
                        nc.gpsimd.dma_start(
                            out=t[:, kt * DH:(kt + 1) * DH],
                            in_=win[(nm, l)][kt * P:(kt + 1) * P, :])
                    wt[(nm, l)] = t
                for nm in ("b1", "b2"):
                    t = cp.tile([P, NKT2], f32, name=f"{nm}{l}_t")
                    for mo in range(NKT2):
                        nc.gpsimd.dma_start(
                            out=t[:, mo:mo + 1],
                            in_=win[(nm, l)][mo * P:(mo + 1) * P, :])
                    wt[(nm, l)] = t

            # resident adjacency: the whole per-core A.T in fp8
            acache = cp.tile([P, KC2, NPC2], f8, name="acache")

            # layer-boundary activation tables: one shared piece per
            # (layer, MLP group); piece gi holds rows [g0*P, (g0+gm)*P) of
            # every core's padded shard, concatenated by core
            zloc = [dp.tile([NPC2, DH], f8, name=f"zloc{l}")
                    for l in range(L - 1)]
            zfp = [[dp.tile([NCORES * gm * P, DH], f8,
                            name=f"zfp{l}_{gi}", addr_space="Shared")
                    for gi, (g0, gm) in enumerate(GROUPS)]
                   for l in range(L - 1)]

            # ---------------- layers ----------------
            for l in range(L):
                din = DIN if l == 0 else DH
                nkt = din // P
                last = (l == L - 1)

                # activation table -> SBUF, node-major chunks:
                # zsb[p, k, f] = z_pad[k*128+p, f]
                zsb = zsp.tile([P, KC2, din], f8, name=f"zsb_{l}", tag="zsb")
                if l == 0:
                    nc.scalar.dma_start(
                        out=zsb[:, :, :].rearrange("p k f -> p (k f)"),
                        in_=xzin[:, :])
                else:
                    # per (piece, core) aligned loads, piece-availability
                    # order, alternating issue queues
                    for gi, (g0, gm) in enumerate(GROUPS):
                        for q in range(NCORES):
                            k0 = q * MT + g0
                            eng = nc.scalar if q % 2 == 0 else nc.sync
                            eng.dma_start(
                                out=zsb[:, k0:k0 + gm, :],
                                in_=zfp[l - 1][gi]
                                    [q * gm * P:(q + 1) * gm * P, :]
                                    .rearrange("(k p) f -> p k f", p=P))

                # --- aggregation: h.T = z.T @ Aloc.T, fp8 DoubleRow pairs,
                # k-outer so each stationary zsb slice is loaded once
                hps = [hpsum.tile([P, nkt * 512], f32,
                                  name=f"hps{gi}_{l}", tag=f"hps{gi}")
                       for gi in range(len(NG))]
                porder = list(range(PAIRS)) if l == 0 else pair_order

                def agg_phase(gis, stream):
                    for pi, p in enumerate(porder):
                        if stream and p % (ABATCH // 2) == 0:
                            b = p // (ABATCH // 2)
                            nc.sync.dma_start(
                                out=acache[:, b * ABATCH:(b + 1) * ABATCH, :]
                                    .rearrange("p k s -> p (k s)"),
                                in_=a8in[b, :, :])
                        for mf in range(nkt):
                            for gi in gis:
                                n0, nn = NG[gi]
                                nc.tensor.matmul(
                                    out=hps[gi][:, mf * 512: mf * 512 + nn],
                                    lhsT=zsb[:, 2 * p:2 * p + 2,
                                             mf * P:(mf + 1) * P],
                                    rhs=acache[:, 2 * p:2 * p + 2, n0:n0 + nn],
                                    start=(pi == 0),
                                    stop=(pi == PAIRS - 1),
                                    perf_mode=DR,
                                )

                if l == 0:
                    agg_phase([0, 1, 2], stream=True)
                else:
                    agg_phase([0], stream=False)

                # --- MLP per dst group (aligned with NG: 512 rows each)
                hhi = [hp.tile([P, NPC2], bf, name=f"hhi{mf}_{l}", tag=f"hhi{mf}")
                       for mf in range(nkt)]
                hlo = [hp.tile([P, NPC2], bf, name=f"hlo{mf}_{l}", tag=f"hlo{mf}")
                       for mf in range(nkt)]
                zT = [zp.tile([P, NPC2], f32 if last else bf,
                              name=f"zT{mo}_{l}",
                              tag=f"zT{mo}{'f' if last else 'b'}")
                      for mo in range(NKT2)]
                def mlp_group(gi):
                    g0, gm = GROUPS[gi]
                    rows = gm * P
                    r0 = g0 * P
                    # split this group's h into hi/lo bf16
                    for mf in range(nkt):
                        nc.vector.tensor_copy(
                            out=hhi[mf][:, r0:r0 + rows],
                            in_=hps[gi][:, mf * 512: mf * 512 + rows])
                        nc.vector.tensor_tensor(
                            out=hlo[mf][:, r0:r0 + rows],
                            in0=hps[gi][:, mf * 512: mf * 512 + rows],
                            in1=hhi[mf][:, r0:r0 + rows],
                            op=SUB)

                    combos1 = [("w1h", hhi), ("w1h", hlo), ("w1l", hhi)][:NSPLIT]
                    s1h, s1l = [], []
                    for mo in range(NKT2):
                        p1 = mlppool.tile([P, 512], f32,
                                          name=f"p1_{l}_{g0}_{mo}", tag="mlp")
                        tot = len(combos1) * nkt
                        step = 0
                        for (wn, ht) in combos1:
                            for kt in range(nkt):
                                nc.tensor.matmul(
                                    out=p1[:, :rows],
                                    lhsT=wt[(wn, l)][:, kt * DH + mo * P: kt * DH + (mo + 1) * P],
                                    rhs=ht[kt][:, r0:r0 + rows],
                                    start=(step == 0), stop=(step == tot - 1))
                                step += 1
                        s1f = sp.tile([P, 512], f32, name=f"s1f_{l}_{g0}_{mo}", tag="s1f")
                        nc.scalar.activation(
                            out=s1f[:, :rows], in_=p1[:, :rows], func=RELU,
                            bias=wt[("b1", l)][:, mo:mo + 1])
                        sh = sp.tile([P, 512], bf, name=f"s1h_{l}_{g0}_{mo}", tag=f"s1h{mo}")
                        nc.vector.tensor_copy(out=sh[:, :rows], in_=s1f[:, :rows])
                        sl = sp.tile([P, 512], bf, name=f"s1l_{l}_{g0}_{mo}", tag=f"s1l{mo}")
                        nc.vector.tensor_tensor(
                            out=sl[:, :rows], in0=s1f[:, :rows], in1=sh[:, :rows], op=SUB)
                        s1h.append(sh)
                        s1l.append(sl)
                    combos2 = [("w2h", s1h), ("w2h", s1l), ("w2l", s1h)][:NSPLIT]
                    for mo in range(NKT2):
                        p2 = mlppool.tile([P, 512], f32,
                                          name=f"p2_{l}_{g0}_{mo}", tag="mlp")
                        tot = len(combos2) * NKT2
                        step = 0
                        for (wn, st) in combos2:
                            for kt in range(NKT2):
                                nc.tensor.matmul(
                                    out=p2[:, :rows],
                                    lhsT=wt[(wn, l)][:, kt * DH + mo * P: kt * DH + (mo + 1) * P],
                                    rhs=st[kt][:, :rows],
                                    start=(step == 0), stop=(step == tot - 1))
                                step += 1
                        nc.scalar.activation(
                            out=zT[mo][:, r0:r0 + rows], in_=p2[:, :rows], func=RELU,
                            bias=wt[("b2", l)][:, mo:mo + 1])

                    # transpose back to row-major + store
                    ident = identf_t if last else identb_t
                    for m in range(g0, g0 + gm):
                        rows_m = min(P, NPC - m * P) if last else P
                        tp = mlppool.tile([P, NKT2 * P], f32 if last else bf,
                                          name=f"tp_{l}_{m}", tag="mlp")
                        for mo in range(NKT2):
                            nc.tensor.transpose(
                                out=tp[:, mo * P:(mo + 1) * P],
                                in_=zT[mo][:, m * P:(m + 1) * P],
                                identity=ident[:])
                        zr = zrp.tile([P, NKT2 * P], f32 if last else f8,
                                      name=f"zr_{l}_{m}", tag="zr")
                        nc.vector.tensor_copy(out=zr[:], in_=tp[:])
                        dst = zout if last else zloc[l]
                        nc.sync.dma_start(
                            out=dst[m * P: m * P + rows_m, :],
                            in_=zr[:rows_m, :])

                    # launch this group's gather piece as soon as its
                    # stores land; the next layer starts on piece 0
                    if not last:
                        nc.gpsimd.collective_compute(
                            "AllGather",
                            mybir.AluOpType.bypass,
                            replica_groups=[list(range(NCORES))],
                            ins=[zloc[l][r0:r0 + rows, :].opt()],
                            outs=[zfp[l][gi][:, :].opt()],
                        )

                mlp_group(0)
                if l != 0:
                    agg_phase([1, 2], stream=False)
                mlp_group(1)
                mlp_group(2)

    nc.compile()
    return nc


# --------------------------------------------------------------------------
# entry point
# --------------------------------------------------------------------------

def _make_in_maps(inputs, cfg, a8):
    DIN, DH, L = cfg["DIN"], cfg["DH"], cfg["L"]
    xz = _prep_xz(inputs["x"], DIN, cfg["NPC"], cfg["NPC2"], cfg["KC2"])
    identb = np.eye(P, dtype=np.float32).astype(BF16)
    identf = np.eye(P, dtype=np.float32)

    shared = {"xz": xz, "identb": identb, "identf": identf}
    for l in range(L):
        w1 = np.asarray(inputs[f"w1_{l}"], dtype=np.float32)
        w2 = np.asarray(inputs[f"w2_{l}"], dtype=np.float32)
        w1h = w1.astype(BF16)
        w2h = w2.astype(BF16)
        shared[f"w1h_{l}"] = w1h
        shared[f"w1l_{l}"] = (w1 - w1h.astype(np.float32)).astype(BF16)
        shared[f"w2h_{l}"] = w2h
        shared[f"w2l_{l}"] = (w2 - w2h.astype(np.float32)).astype(BF16)
        shared[f"b1_{l}"] = np.asarray(
            inputs[f"b1_{l}"], dtype=np.float32).reshape(DH, 1)
        shared[f"b2_{l}"] = np.asarray(
            inputs[f"b2_{l}"], dtype=np.float32).reshape(DH, 1)

    in_maps = []
    for c in range(NCORES):
        m = dict(shared)
        m["a8"] = a8[c]
        in_maps.append(m)
    return in_maps


def get_program(inputs):
    """Build (or fetch cached) the bass program + per-core input maps."""
    cfg = _config(inputs)
    a8 = _prep_a8(inputs["edge_index"], cfg["N"], cfg["NPC"], cfg["NPC2"],
                  cfg["KC2"])
    key = (cfg["N"], cfg["DIN"], cfg["DH"], cfg["L"], NSPLIT)
    if key not in _BUILD_CACHE:
        _BUILD_CACHE[key] = _build(
            cfg["N"], cfg["DIN"], cfg["DH"], cfg["L"],
            cfg["NPC"], cfg["MT"], cfg["NPC2"], cfg["N2"], cfg["KC2"])
    nc = _BUILD_CACHE[key]
    in_maps = _make_in_maps(inputs, cfg, a8)
    return nc, in_maps, cfg


def kernel(**inputs):
    nc, in_maps, cfg = get_program(inputs)

    if os.environ.get("KERNEL_USE_SIM"):
        from concourse.bass_interp import MultiCoreSim
        sim = MultiCoreSim(nc, num_cores=NCORES)
        cores = list(sim.cores.values())
        for cid, cs in enumerate(cores):
            for name, val in in_maps[cid].items():
                cs.tensor(name)[:] = val
        sim.simulate(check_with_hw=False)
        parts = [np.asarray(cs.tensor("zout")) for cs in cores]
    else:
        from concourse import bass_utils
        res = bass_utils.run_bass_kernel_spmd(
            nc, in_maps, core_ids=list(range(NCORES)),
            trace=bool(os.environ.get("KERNEL_TRACE")),
        )
        kernel.last_results = res
        parts = [res.results[c]["zout"] for c in range(NCORES)]

    out = np.concatenate(parts, axis=0).astype(np.float32)
    return out
